# revision 2
# baseline (speedup 1.0000x reference)
"""ColPali MQA attention block on 8 Trainium2 NeuronCores.

The reference contains the ColPali reshape quirk: the attention output
[B, H, L, 1, D] is reshaped row-major straight to [B, L, H*D], which mixes
heads and positions.  Output row l' therefore depends ONLY on head
h = l'//256, gathering positions (l'%256)*8 + j for j in 0..7:

    Y[b, l', e] = sum_{j,d} O[b, l'//256, (l'%256)*8+j, d] * Wo[e, j*256+d]

Sharding: core c -> batch b=c//4 and heads {h0, h0+1} with h0=2*(c%4).
Q projection + attention for the core's 2 heads over the full sequence and
o_proj for output rows [512*(c%4), +512) are per-core.  K/V projection is
split across the 4-core batch group: core c computes K/V (with K-RoPE) for
positions [512*(c%4), +512) only, then the group runs an AllGather (bf16,
512KB/rank) that lands full K/V on every core while the Q projection keeps
the PE busy.  Per-core outputs are disjoint [512, 2048] slices of the
[2, 2048, 2048] output -> no further cross-core communication.

All inputs are pre-cast to bf16 on the host (matmuls run bf16 anyway), so
DMA traffic is halved and SBUF tiles are filled directly with no staging /
conversion passes.

Layouts (contraction dim always on SBUF partitions; zero on-device
transposes):
  - q, k produced transposed ([D, L]) by making W the stationary operand.
  - v produced natural ([L, D]) by making X the stationary operand.
  - scores computed transposed: S^T[lk, lq] = k @ q_h^T, so the exp output
    P^T[lk, lq] directly feeds O^T[d, lq] = v^T @ P^T as moving operand.
  - softmax row sums over lk (= partitions): the 16 P^T tiles are first
    pair-reduced twice on the vector engine (bf16), then a single all-ones
    [128,128] stationary matmul pass over the 4 partial tiles lands the
    sums pre-broadcast across all 128 partitions; reciprocal_approx_fast
    gives the scale tile.  The normalize multiply writes through a
    (u w)->(w u) access pattern that performs the ColPali gather for free,
    producing G[c, r] = O^T[d, r*8+j] (c = j*256+d) which is directly the
    stationary operand of o_proj.
"""

import numpy as np
import ml_dtypes

import concourse.mybir as mybir
import concourse.tile as tile
from concourse import bacc
from concourse.bass_utils import run_bass_kernel_spmd

F32 = mybir.dt.float32
BF16 = mybir.dt.bfloat16
AF = mybir.ActivationFunctionType
OP = mybir.AluOpType

B, L, H, D, E = 2, 2048, 8, 256, 2048
HD = H * D  # 2048
P = 128
EC = E // P  # 16 e-chunks
LT = L // P  # 16 l-tiles
SCALING = D ** -0.5  # 1/16
N_CORES = 8
RG = [[0, 1, 2, 3], [4, 5, 6, 7]]  # batch groups share K/V


def build_program():
    nc = bacc.Bacc("TRN2", target_bir_lowering=False, debug=False,
                   num_devices=N_CORES)

    xt = nc.dram_tensor("xt", [E, L], BF16, kind="ExternalInput").ap()
    xto = nc.dram_tensor("xto", [E, 512], BF16, kind="ExternalInput").ap()
    cost = nc.dram_tensor("cost", [D, L], BF16, kind="ExternalInput").ap()
    sint = nc.dram_tensor("sint", [D, L], BF16, kind="ExternalInput").ap()
    costo = nc.dram_tensor("costo", [D, 512], BF16,
                           kind="ExternalInput").ap()
    sinto = nc.dram_tensor("sinto", [D, 512], BF16,
                           kind="ExternalInput").ap()
    wqt = nc.dram_tensor("wqt", [E, 2 * D], BF16, kind="ExternalInput").ap()
    wkt = nc.dram_tensor("wkt", [E, D], BF16, kind="ExternalInput").ap()
    wvt = nc.dram_tensor("wvt", [E, D], BF16, kind="ExternalInput").ap()
    wot = nc.dram_tensor("wot", [HD, E], BF16, kind="ExternalInput").ap()
    out = nc.dram_tensor("out", [4 * P, E], F32, kind="ExternalOutput").ap()

    with tile.TileContext(nc) as tc:
        with tc.tile_pool(name="res", bufs=1) as res, \
             tc.tile_pool(name="dram", bufs=1, space="DRAM") as dram:
            kT = [res.tile([P, L], BF16, tag=f"kT{i}", name=f"kT{i}")
                  for i in range(2)]
            v_bf = [res.tile([P, D], BF16, tag=f"v{i}", name=f"v{i}")
                    for i in range(LT)]
            # q^T for the core's two heads: 4 dq-tiles x [128, L]
            qT = [res.tile([P, L], BF16, tag=f"qT{i}", name=f"qT{i}")
                  for i in range(4)]
            # own K/V chunk staging (pre-AllGather)
            kTo = [res.tile([P, 512], BF16, tag=f"kTo{i}", name=f"kTo{i}")
                   for i in range(2)]
            vo = [res.tile([P, D], BF16, tag=f"vo{i}", name=f"vo{i}")
                  for i in range(4)]
            ones128 = res.tile([P, P], BF16, tag="ones128", name="ones128")
            nc.vector.memset(ones128[:], 1.0)

            # K/V AllGather bounce buffers (HBM).  in: [128, 2048] bf16 =
            # 512KB (kTo halves at cols 0:1024, vo tiles at 1024:2048);
            # out: 4 rank shards concatenated on axis 0.
            in_bounce = dram.tile([P, 2048], BF16, name="in_bounce")
            out_bounce = dram.tile([4 * P, 2048], BF16, name="out_bounce")

            def _rope(p0, p1, out0, out1, cs, pool, tag):
                cos0, cos1, sin0, sin1 = cs
                ta = pool.tile([P, 512], F32, tag="ropetmp", bufs=4,
                               name=f"ta{tag}")
                tb = pool.tile([P, 512], F32, tag="ropetmp", bufs=4,
                               name=f"tb{tag}")
                nc.vector.tensor_tensor(ta[:], p0[:], cos0[:], OP.mult)
                nc.vector.tensor_tensor(tb[:], p1[:], sin0[:], OP.mult)
                nc.vector.tensor_tensor(out0, ta[:], tb[:], OP.subtract)
                tc2 = pool.tile([P, 512], F32, tag="ropetmp", bufs=4,
                                name=f"tc{tag}")
                td = pool.tile([P, 512], F32, tag="ropetmp", bufs=4,
                               name=f"td{tag}")
                nc.vector.tensor_tensor(tc2[:], p1[:], cos1[:], OP.mult)
                nc.vector.tensor_tensor(td[:], p0[:], sin1[:], OP.mult)
                nc.vector.tensor_tensor(out1, tc2[:], td[:], OP.add)

            # ---------------- Phase 1: projections + RoPE ----------------
            with tc.tile_pool(name="proj", bufs=1) as proj, \
                 tc.tile_pool(name="proj_ps", space="PSUM", bufs=1) as pps:
                xt_bf = [proj.tile([P, L], BF16, tag=f"xt{e}",
                                   name=f"xtbf{e}") for e in range(EC)]
                xto_bf = [proj.tile([P, 512], BF16, tag=f"xto{e}",
                                    name=f"xtobf{e}") for e in range(EC)]
                wkt_bf = [proj.tile([P, D], BF16, tag=f"wkt{e}",
                                    name=f"wktbf{e}") for e in range(EC)]
                wvt_bf = [proj.tile([P, D], BF16, tag=f"wvt{e}",
                                    name=f"wvtbf{e}") for e in range(EC)]
                wqt_bf = [proj.tile([P, 2 * D], BF16, tag=f"wqt{e}",
                                    name=f"wqtbf{e}") for e in range(EC)]

                # --- Phase 1a: own-chunk K/V projection, then AllGather ---
                for e in range(EC):
                    nc.sync.dma_start(out=wkt_bf[e][:],
                                      in_=wkt[e * P:(e + 1) * P, :])
                    nc.sync.dma_start(out=xto_bf[e][:],
                                      in_=xto[e * P:(e + 1) * P, :])
                csko = []
                for srcd in (costo, sinto):
                    for half in range(2):
                        t = proj.tile([P, 512], BF16, tag="csko", bufs=4,
                                      name=f"csko{len(csko)}")
                        nc.sync.dma_start(
                            out=t[:], in_=srcd[half * P:(half + 1) * P, :])
                        csko.append(t)
                csko = [csko[0], csko[1], csko[2], csko[3]]  # c0 c1 s0 s1
                for e in range(EC):
                    nc.sync.dma_start(out=wvt_bf[e][:],
                                      in_=wvt[e * P:(e + 1) * P, :])

                pk0 = pps.tile([P, 512], F32, tag="pk", bufs=2, name="pk0")
                pk1 = pps.tile([P, 512], F32, tag="pk", bufs=2, name="pk1")
                for e in range(EC):
                    st, sp = (e == 0), (e == EC - 1)
                    nc.tensor.matmul(pk0[:], wkt_bf[e][:, 0:P], xto_bf[e][:],
                                     start=st, stop=sp)
                    nc.tensor.matmul(pk1[:], wkt_bf[e][:, P:2 * P],
                                     xto_bf[e][:], start=st, stop=sp)
                _rope(pk0, pk1, kTo[0][:], kTo[1][:],
                      (csko[0], csko[1], csko[2], csko[3]), proj, "k")

                for lt in range(4):
                    pv = pps.tile([P, D], F32, tag="pv", bufs=2,
                                  name=f"pv{lt}")
                    for e in range(EC):
                        nc.tensor.matmul(pv[:],
                                         xto_bf[e][:, lt * P:(lt + 1) * P],
                                         wvt_bf[e][:],
                                         start=(e == 0), stop=(e == EC - 1))
                    nc.vector.tensor_copy(vo[lt][:], pv[:])

                # bounce out, AllGather within the 4-core batch group,
                # scatter the gathered shards back into the full K/V tiles.
                nc.gpsimd.dma_start(out=in_bounce[:, 0:512], in_=kTo[0][:])
                nc.gpsimd.dma_start(out=in_bounce[:, 512:1024],
                                    in_=kTo[1][:])
                for i in range(4):
                    nc.gpsimd.dma_start(
                        out=in_bounce[:, 1024 + i * D:1024 + (i + 1) * D],
                        in_=vo[i][:])
                nc.gpsimd.collective_compute(
                    "AllGather",
                    mybir.AluOpType.bypass,
                    replica_groups=RG,
                    ins=[in_bounce.opt()],
                    outs=[out_bounce.opt()],
                )
                for r in range(4):
                    rows = out_bounce[r * P:(r + 1) * P, :]
                    sl = slice(r * 512, (r + 1) * 512)
                    nc.gpsimd.dma_start(out=kT[0][:, sl], in_=rows[:, 0:512])
                    nc.gpsimd.dma_start(out=kT[1][:, sl],
                                        in_=rows[:, 512:1024])
                    for i in range(4):
                        nc.gpsimd.dma_start(
                            out=v_bf[4 * r + i][:],
                            in_=rows[:, 1024 + i * D:1024 + (i + 1) * D])

                # --- Phase 1b: Q projection + RoPE over the full sequence,
                # overlapping the AllGather ---
                for e in range(EC):
                    nc.sync.dma_start(out=wqt_bf[e][:],
                                      in_=wqt[e * P:(e + 1) * P, :])
                for lc in range(4):
                    sl = slice(lc * 512, (lc + 1) * 512)
                    for e in range(EC):
                        nc.sync.dma_start(out=xt_bf[e][:, sl],
                                          in_=xt[e * P:(e + 1) * P, sl])
                    csq = []
                    for srcd in (cost, sint):
                        for half in range(2):
                            t = proj.tile([P, 512], BF16,
                                          tag=f"csq{len(csq)}", bufs=2,
                                          name=f"csq{lc}_{len(csq)}")
                            nc.sync.dma_start(
                                out=t[:],
                                in_=srcd[half * P:(half + 1) * P, sl])
                            csq.append(t)

                    pq = [pps.tile([P, 512], F32, tag=f"pq{j}", bufs=1,
                                   name=f"pq{lc}_{j}") for j in range(4)]
                    for e in range(EC):
                        st, sp = (e == 0), (e == EC - 1)
                        xs = xt_bf[e][:, sl]
                        for j in range(4):
                            nc.tensor.matmul(pq[j][:],
                                             wqt_bf[e][:, j * P:(j + 1) * P],
                                             xs, start=st, stop=sp)
                    _rope(pq[0], pq[1], qT[0][:, sl], qT[1][:, sl],
                          (csq[0], csq[1], csq[2], csq[3]), proj, f"q0{lc}")
                    _rope(pq[2], pq[3], qT[2][:, sl], qT[3][:, sl],
                          (csq[0], csq[1], csq[2], csq[3]), proj, f"q1{lc}")

            # ------------- Phase 2: attention + o_proj -------------
            with tc.tile_pool(name="att", bufs=1) as att, \
                 tc.tile_pool(name="att_ps", space="PSUM", bufs=1) as aps:
                # G[hh][half]: gathered, normalized O^T.  G[c-row, col] with
                # c-row = d within half, column layout j*256 + r.
                G = [[att.tile([P, L], BF16, tag=f"G{hh}{dt}",
                               name=f"G{hh}{dt}") for dt in range(2)]
                     for hh in range(2)]
                wot_bf = [att.tile([P, E], BF16, tag=f"wot{i}",
                                   name=f"wotbf{i}") for i in range(EC)]
                for i in range(EC):
                    nc.sync.dma_start(out=wot_bf[i][:],
                                      in_=wot[i * P:(i + 1) * P, :])

                def o_proj(a_idx):
                    for rh in range(2):
                        rt = a_idx * 2 + rh
                        for eg in range(4):
                            esl = slice(eg * 512, (eg + 1) * 512)
                            py = aps.tile([P, 512], F32, tag="py", bufs=2,
                                          name=f"py{rt}_{eg}")
                            for m in range(EC):
                                lhsT = G[a_idx][m % 2][
                                    :, (m // 2) * 256 + rh * P:
                                       (m // 2) * 256 + rh * P + P]
                                nc.tensor.matmul(py[:], lhsT,
                                                 wot_bf[m][:, esl],
                                                 start=(m == 0),
                                                 stop=(m == EC - 1))
                            ysb = att.tile([P, 512], F32, tag="ysb", bufs=3,
                                           name=f"ysb{rt}_{eg}")
                            nc.scalar.copy(ysb[:], py[:])
                            nc.sync.dma_start(
                                out=out[rt * P:(rt + 1) * P, esl],
                                in_=ysb[:])

                for hh in range(2):
                    qh0, qh1 = qT[2 * hh], qT[2 * hh + 1]
                    for lqc in range(4):
                        qsl = slice(lqc * 512, (lqc + 1) * 512)
                        pt = [att.tile([P, 512], BF16, tag=f"pt{i}", bufs=2,
                                       name=f"pt{hh}_{lqc}_{i}")
                              for i in range(LT)]
                        for lk in range(LT):
                            ps = aps.tile([P, 512], F32, tag="ps", bufs=3,
                                          name=f"ps{hh}_{lqc}_{lk}")
                            nc.tensor.matmul(ps[:],
                                             kT[0][:, lk * P:(lk + 1) * P],
                                             qh0[:, qsl],
                                             start=True, stop=False)
                            nc.tensor.matmul(ps[:],
                                             kT[1][:, lk * P:(lk + 1) * P],
                                             qh1[:, qsl],
                                             start=False, stop=True)
                            nc.scalar.activation(pt[lk][:], ps[:], AF.Exp,
                                                 scale=float(SCALING))
                        # Pair-reduce the 16 P^T tiles twice on DVE (bf16),
                        # so the cross-partition row-sum matmul streams 4
                        # tiles instead of 16.
                        s8 = [att.tile([P, 512], BF16, tag=f"s8_{i}",
                                       bufs=2, name=f"s8{hh}_{lqc}_{i}")
                              for i in range(8)]
                        for i in range(8):
                            nc.vector.tensor_tensor(s8[i][:], pt[2 * i][:],
                                                    pt[2 * i + 1][:], OP.add)
                        s4 = [att.tile([P, 512], BF16, tag=f"s4_{i}",
                                       bufs=2, name=f"s4{hh}_{lqc}_{i}")
                              for i in range(4)]
                        for i in range(4):
                            nc.vector.tensor_tensor(s4[i][:], s8[2 * i][:],
                                                    s8[2 * i + 1][:], OP.add)
                        # attn @ v first (keeps PE busy while DVE finishes
                        # the pair-adds), then the row-sum matmul.
                        po = [None, None]
                        for dt in range(2):
                            po[dt] = aps.tile([P, 512], F32, tag="po",
                                              bufs=2,
                                              name=f"po{hh}_{lqc}_{dt}")
                            for lk in range(LT):
                                nc.tensor.matmul(
                                    po[dt][:],
                                    v_bf[lk][:, dt * P:(dt + 1) * P],
                                    pt[lk][:],
                                    start=(lk == 0), stop=(lk == LT - 1))
                        # Row sums, pre-broadcast over all 128 partitions
                        # by the all-ones stationary operand.
                        prb = aps.tile([P, 512], F32, tag="prb", bufs=1,
                                       name=f"prb{hh}_{lqc}")
                        for i in range(4):
                            nc.tensor.matmul(prb[:], ones128[:], s4[i][:],
                                             start=(i == 0), stop=(i == 3))
                        rb = att.tile([P, 512], F32, tag="rb", bufs=2,
                                      name=f"rb{hh}_{lqc}")
                        nc.vector.reciprocal_approx_fast(rb[:], prb[:])
                        rb_wu = rb.rearrange("p (u w) -> p w u", w=8)
                        for dt in range(2):
                            # normalize + ColPali gather in one op:
                            # G[:, j*256 + 64*lqc + u] = po[:, 8u+j]*rb[:, 8u+j]
                            g_dst = G[hh][dt].rearrange(
                                "p (w r) -> p w r",
                                w=8)[:, :, 64 * lqc:64 * lqc + 64]
                            nc.vector.tensor_tensor(
                                g_dst,
                                po[dt].rearrange("p (u w) -> p w u", w=8),
                                rb_wu, OP.mult)
                    o_proj(hh)

    nc.compile()
    return nc


_NC = None


def _get_nc():
    global _NC
    if _NC is None:
        _NC = build_program()
    return _NC


def make_in_maps(hidden_states, cos, sin, Wq, Wk, Wv, Wo):
    bf = ml_dtypes.bfloat16
    hs = np.asarray(hidden_states, np.float32)
    xt = [np.ascontiguousarray(hs[b].T.astype(bf)) for b in range(B)]
    cost = np.ascontiguousarray(np.asarray(cos, np.float32).T.astype(bf))
    sint = np.ascontiguousarray(np.asarray(sin, np.float32).T.astype(bf))
    wqt = np.ascontiguousarray(np.asarray(Wq, np.float32).T.astype(bf))
    wkt = np.ascontiguousarray(np.asarray(Wk, np.float32).T.astype(bf))
    wvt = np.ascontiguousarray(np.asarray(Wv, np.float32).T.astype(bf))
    wot = np.ascontiguousarray(np.asarray(Wo, np.float32).T.astype(bf))
    in_maps = []
    for c in range(N_CORES):
        b, ql = c // 4, c % 4
        sl = slice(ql * 512, (ql + 1) * 512)
        in_maps.append({
            "xt": xt[b],
            "xto": np.ascontiguousarray(xt[b][:, sl]),
            "cost": cost,
            "sint": sint,
            "costo": np.ascontiguousarray(cost[:, sl]),
            "sinto": np.ascontiguousarray(sint[:, sl]),
            "wqt": np.ascontiguousarray(wqt[:, sl]),
            "wkt": wkt,
            "wvt": wvt,
            "wot": wot,
        })
    return in_maps


def assemble(results):
    y = np.empty((B, L, E), np.float32)
    for c in range(N_CORES):
        b, ql = c // 4, c % 4
        y[b, ql * 512:(ql + 1) * 512, :] = results[c]["out"]
    return y


def kernel(hidden_states, attention_mask, cos, sin, Wq, Wk, Wv, Wo):
    # attention_mask is additive and all-zero per the problem spec; it is
    # accepted for signature compatibility but not shipped to the device.
    nc = _get_nc()
    in_maps = make_in_maps(hidden_states, cos, sin, Wq, Wk, Wv, Wo)
    res = run_bass_kernel_spmd(nc, in_maps, core_ids=list(range(N_CORES)))
    return assemble(res.results)


# revision 4
# speedup vs baseline: 1.0903x; 1.0903x over previous
"""ColPali MQA attention block on 8 Trainium2 NeuronCores.

The reference contains the ColPali reshape quirk: the attention output
[B, H, L, 1, D] is reshaped row-major straight to [B, L, H*D], which mixes
heads and positions.  Output row l' therefore depends ONLY on head
h = l'//256, gathering positions (l'%256)*8 + j for j in 0..7:

    Y[b, l', e] = sum_{j,d} O[b, l'//256, (l'%256)*8+j, d] * Wo[e, j*256+d]

Sharding: core c -> batch b=c//4 and heads {h0, h0+1} with h0=2*(c%4).
Q projection + attention for the core's 2 heads over the full sequence and
o_proj for output rows [512*(c%4), +512) are per-core.  K/V projection is
split across the 4-core batch group: core c computes K/V (with K-RoPE) for
positions [512*(c%4), +512) only, then two AllGathers (k then v, bf16,
256KB/rank each) land full K/V on every core while the Q projection keeps
the PE busy.  Per-core outputs are disjoint [512, 2048] slices of the
[2, 2048, 2048] output -> no further cross-core communication.

All inputs are pre-cast to bf16 AND pre-packed tile-major on the host, so
every device DMA is one fat contiguous transfer (2-16KB per partition
line) straight into its SBUF resident layout -- no staging, no on-device
conversion, no small-line DMA inefficiency.

Layouts (contraction dim always on SBUF partitions; zero on-device
transposes):
  - q, k produced transposed ([D, L]) by making W the stationary operand.
  - v produced natural ([L, D]) by making X the stationary operand.
  - scores computed transposed: S^T[lk, lq] = k @ q_h^T, so the exp output
    P^T[lk, lq] directly feeds O^T[d, lq] = v^T @ P^T as moving operand.
  - softmax row sums over lk (= partitions): the 16 P^T tiles are first
    pair-reduced twice on the vector engine (bf16), then a single all-ones
    [128,128] stationary matmul pass over the 4 partial tiles lands the
    sums pre-broadcast across all 128 partitions; reciprocal_approx_fast
    gives the scale tile.  The normalize multiply writes through a
    (u w)->(w u) access pattern that performs the ColPali gather for free,
    producing G[c, r] = O^T[d, r*8+j] (c = j*256+d) which is directly the
    stationary operand of o_proj.
"""

import numpy as np
import ml_dtypes

import concourse.mybir as mybir
import concourse.tile as tile
from concourse import bacc
from concourse.bass_utils import run_bass_kernel_spmd

F32 = mybir.dt.float32
BF16 = mybir.dt.bfloat16
AF = mybir.ActivationFunctionType
OP = mybir.AluOpType

B, L, H, D, E = 2, 2048, 8, 256, 2048
HD = H * D  # 2048
P = 128
EC = E // P  # 16 e-chunks
LT = L // P  # 16 l-tiles
SCALING = D ** -0.5  # 1/16
N_CORES = 8
RG = [[0, 1, 2, 3], [4, 5, 6, 7]]  # batch groups share K/V


def build_program():
    nc = bacc.Bacc("TRN2", target_bir_lowering=False, debug=False,
                   num_devices=N_CORES)

    # Tile-major packed inputs (see make_in_maps for the host-side layout).
    xt = nc.dram_tensor("xt", [P, 4 * EC * 512], BF16,
                        kind="ExternalInput").ap()
    xto = nc.dram_tensor("xto", [P, EC * 512], BF16,
                         kind="ExternalInput").ap()
    cs = nc.dram_tensor("cs", [P, 4 * 2048], BF16, kind="ExternalInput").ap()
    cso = nc.dram_tensor("cso", [P, 2048], BF16, kind="ExternalInput").ap()
    wkv = nc.dram_tensor("wkv", [P, 2 * EC * D], BF16,
                         kind="ExternalInput").ap()
    wqt = nc.dram_tensor("wqt", [P, EC * 512], BF16,
                         kind="ExternalInput").ap()
    wot = nc.dram_tensor("wot", [P, EC * E], BF16, kind="ExternalInput").ap()
    out = nc.dram_tensor("out", [4 * P, E], F32, kind="ExternalOutput").ap()

    with tile.TileContext(nc) as tc:
        with tc.tile_pool(name="res", bufs=1) as res, \
             tc.tile_pool(name="dram", bufs=1, space="DRAM") as dram:
            # K/V resident layout, one fat tile: rank r owns columns
            # [r*2048, (r+1)*2048) = [kT0(512) | kT1(512) | v0..v3(4x256)].
            kv_sb = res.tile([P, 4 * 2048], BF16, tag="kv", name="kv_sb")

            def KT(i, lk):
                base = (lk // 4) * 2048 + i * 512 + (lk % 4) * P
                return kv_sb[:, base:base + P]

            def VV(lk, dt):
                base = (lk // 4) * 2048 + 1024 + (lk % 4) * D + dt * P
                return kv_sb[:, base:base + P]

            # q^T for the core's two heads: 4 dq-tiles x [128, L]
            qT = [res.tile([P, L], BF16, tag=f"qT{i}", name=f"qT{i}")
                  for i in range(4)]
            # own K/V chunk staging (pre-AllGather)
            kTo = res.tile([P, 1024], BF16, tag="kTo", name="kTo")
            vo = res.tile([P, 1024], BF16, tag="vo", name="vo")
            ones128 = res.tile([P, P], BF16, tag="ones128", name="ones128")
            nc.vector.memset(ones128[:], 1.0)

            # K/V AllGather bounce buffers (HBM).
            in_k = dram.tile([P, 1024], BF16, name="in_k")
            out_k = dram.tile([4 * P, 1024], BF16, name="out_k")
            in_v = dram.tile([P, 1024], BF16, name="in_v")
            out_v = dram.tile([4 * P, 1024], BF16, name="out_v")

            def _rope(p0, p1, out0, out1, cs4, pool, tag):
                cos0, cos1, sin0, sin1 = cs4
                ta = pool.tile([P, 512], F32, tag="ropetmp", bufs=4,
                               name=f"ta{tag}")
                tb = pool.tile([P, 512], F32, tag="ropetmp", bufs=4,
                               name=f"tb{tag}")
                nc.vector.tensor_tensor(ta[:], p0[:], cos0, OP.mult)
                nc.vector.tensor_tensor(tb[:], p1[:], sin0, OP.mult)
                nc.vector.tensor_tensor(out0, ta[:], tb[:], OP.subtract)
                tc2 = pool.tile([P, 512], F32, tag="ropetmp", bufs=4,
                                name=f"tc{tag}")
                td = pool.tile([P, 512], F32, tag="ropetmp", bufs=4,
                               name=f"td{tag}")
                nc.vector.tensor_tensor(tc2[:], p1[:], cos1, OP.mult)
                nc.vector.tensor_tensor(td[:], p0[:], sin1, OP.mult)
                nc.vector.tensor_tensor(out1, tc2[:], td[:], OP.add)

            # ---------------- Phase 1: projections + RoPE ----------------
            with tc.tile_pool(name="proj", bufs=1) as proj, \
                 tc.tile_pool(name="proj_ps", space="PSUM", bufs=1) as pps:
                wkv_sb = proj.tile([P, 2 * EC * D], BF16, tag="wkv",
                                   name="wkv_sb")
                xto_sb = proj.tile([P, EC * 512], BF16, tag="xto",
                                   name="xto_sb")
                cso_sb = proj.tile([P, 2048], BF16, tag="cso", name="cso_sb")
                wqt_sb = proj.tile([P, EC * 512], BF16, tag="wqt",
                                   name="wqt_sb")
                xt_sb = proj.tile([P, 4 * EC * 512], BF16, tag="xt",
                                  name="xt_sb")
                cs_sb = proj.tile([P, 4 * 2048], BF16, tag="cs",
                                  name="cs_sb")

                def WK(e, i):
                    return wkv_sb[:, e * D + i * P:e * D + (i + 1) * P]

                def WV(e):
                    return wkv_sb[:, EC * D + e * D:EC * D + (e + 1) * D]

                def XTO(e):
                    return xto_sb[:, e * 512:(e + 1) * 512]

                def XS(lc, e):
                    base = lc * EC * 512 + e * 512
                    return xt_sb[:, base:base + 512]

                def WQ(e, j):
                    return wqt_sb[:, e * 512 + j * P:e * 512 + (j + 1) * P]

                def CS(lc, s, h):
                    base = lc * 2048 + s * 1024 + h * 512
                    return cs_sb[:, base:base + 512]

                # --- phase-1a DMAs: own-chunk K/V inputs, fat transfers ---
                nc.sync.dma_start(out=wkv_sb[:], in_=wkv[:, :])
                nc.sync.dma_start(out=xto_sb[:], in_=xto[:, :])
                nc.sync.dma_start(out=cso_sb[:], in_=cso[:, :])
                nc.sync.dma_start(out=wqt_sb[:], in_=wqt[:, :])

                # K projection (own 512 positions) + RoPE.
                pk0 = pps.tile([P, 512], F32, tag="pk", bufs=2, name="pk0")
                pk1 = pps.tile([P, 512], F32, tag="pk", bufs=2, name="pk1")
                for e in range(EC):
                    st, sp = (e == 0), (e == EC - 1)
                    nc.tensor.matmul(pk0[:], WK(e, 0), XTO(e),
                                     start=st, stop=sp)
                    nc.tensor.matmul(pk1[:], WK(e, 1), XTO(e),
                                     start=st, stop=sp)
                _rope(pk0, pk1, kTo[:, 0:512], kTo[:, 512:1024],
                      (cso_sb[:, 0:512], cso_sb[:, 512:1024],
                       cso_sb[:, 1024:1536], cso_sb[:, 1536:2048]),
                      proj, "k")
                nc.gpsimd.dma_start(out=in_k[:], in_=kTo[:])
                nc.gpsimd.collective_compute(
                    "AllGather", mybir.AluOpType.bypass, replica_groups=RG,
                    ins=[in_k.opt()], outs=[out_k.opt()])

                # V projection (own 512 positions).
                for lt in range(4):
                    pv = pps.tile([P, D], F32, tag="pv", bufs=2,
                                  name=f"pv{lt}")
                    for e in range(EC):
                        nc.tensor.matmul(pv[:],
                                         XTO(e)[:, lt * P:(lt + 1) * P],
                                         WV(e),
                                         start=(e == 0), stop=(e == EC - 1))
                    nc.vector.tensor_copy(vo[:, lt * D:(lt + 1) * D], pv[:])
                nc.gpsimd.dma_start(out=in_v[:], in_=vo[:])
                nc.gpsimd.collective_compute(
                    "AllGather", mybir.AluOpType.bypass, replica_groups=RG,
                    ins=[in_v.opt()], outs=[out_v.opt()])
                # Scatter the gathered shards into the resident K/V layout.
                for r in range(4):
                    nc.gpsimd.dma_start(
                        out=kv_sb[:, r * 2048:r * 2048 + 1024],
                        in_=out_k[r * P:(r + 1) * P, :])
                for r in range(4):
                    nc.gpsimd.dma_start(
                        out=kv_sb[:, r * 2048 + 1024:r * 2048 + 2048],
                        in_=out_v[r * P:(r + 1) * P, :])

                # --- Phase 1b: Q projection + RoPE over the full sequence,
                # overlapping the AllGathers ---
                for lc in range(4):
                    sl = slice(lc * 512, (lc + 1) * 512)
                    nc.sync.dma_start(
                        out=xt_sb[:, lc * EC * 512:(lc + 1) * EC * 512],
                        in_=xt[:, lc * EC * 512:(lc + 1) * EC * 512])
                    nc.sync.dma_start(
                        out=cs_sb[:, lc * 2048:(lc + 1) * 2048],
                        in_=cs[:, lc * 2048:(lc + 1) * 2048])

                    pq = [pps.tile([P, 512], F32, tag=f"pq{j}", bufs=1,
                                   name=f"pq{lc}_{j}") for j in range(4)]
                    for e in range(EC):
                        st, sp = (e == 0), (e == EC - 1)
                        xs = XS(lc, e)
                        for j in range(4):
                            nc.tensor.matmul(pq[j][:], WQ(e, j), xs,
                                             start=st, stop=sp)
                    _rope(pq[0], pq[1], qT[0][:, sl], qT[1][:, sl],
                          (CS(lc, 0, 0), CS(lc, 0, 1),
                           CS(lc, 1, 0), CS(lc, 1, 1)), proj, f"q0{lc}")
                    _rope(pq[2], pq[3], qT[2][:, sl], qT[3][:, sl],
                          (CS(lc, 0, 0), CS(lc, 0, 1),
                           CS(lc, 1, 0), CS(lc, 1, 1)), proj, f"q1{lc}")

            # ------------- Phase 2: attention + o_proj -------------
            with tc.tile_pool(name="att", bufs=1) as att, \
                 tc.tile_pool(name="att_ps", space="PSUM", bufs=1) as aps:
                # G[hh][half]: gathered, normalized O^T.  G[c-row, col] with
                # c-row = d within half, column layout j*256 + r.
                G = [[att.tile([P, L], BF16, tag=f"G{hh}{dt}",
                               name=f"G{hh}{dt}") for dt in range(2)]
                     for hh in range(2)]
                wot_sb = att.tile([P, EC * E], BF16, tag="wot",
                                  name="wot_sb")
                for i in range(4):
                    qe = EC * E // 4
                    nc.sync.dma_start(out=wot_sb[:, i * qe:(i + 1) * qe],
                                      in_=wot[:, i * qe:(i + 1) * qe])

                def WOT(m, eg):
                    base = m * E + eg * 512
                    return wot_sb[:, base:base + 512]

                def o_proj(a_idx):
                    for rh in range(2):
                        rt = a_idx * 2 + rh
                        for eg in range(4):
                            py = aps.tile([P, 512], F32, tag="py", bufs=2,
                                          name=f"py{rt}_{eg}")
                            for m in range(EC):
                                lhsT = G[a_idx][m % 2][
                                    :, (m // 2) * 256 + rh * P:
                                       (m // 2) * 256 + rh * P + P]
                                nc.tensor.matmul(py[:], lhsT, WOT(m, eg),
                                                 start=(m == 0),
                                                 stop=(m == EC - 1))
                            ysb = att.tile([P, 512], F32, tag="ysb", bufs=3,
                                           name=f"ysb{rt}_{eg}")
                            nc.scalar.copy(ysb[:], py[:])
                            nc.sync.dma_start(
                                out=out[rt * P:(rt + 1) * P,
                                        eg * 512:(eg + 1) * 512],
                                in_=ysb[:])

                for hh in range(2):
                    qh0, qh1 = qT[2 * hh], qT[2 * hh + 1]
                    for lqc in range(4):
                        qsl = slice(lqc * 512, (lqc + 1) * 512)
                        pt = [att.tile([P, 512], BF16, tag=f"pt{i}", bufs=2,
                                       name=f"pt{hh}_{lqc}_{i}")
                              for i in range(LT)]
                        for lk in range(LT):
                            ps = aps.tile([P, 512], F32, tag="ps", bufs=3,
                                          name=f"ps{hh}_{lqc}_{lk}")
                            nc.tensor.matmul(ps[:], KT(0, lk), qh0[:, qsl],
                                             start=True, stop=False)
                            nc.tensor.matmul(ps[:], KT(1, lk), qh1[:, qsl],
                                             start=False, stop=True)
                            nc.scalar.activation(pt[lk][:], ps[:], AF.Exp,
                                                 scale=float(SCALING))
                        # Pair-reduce the 16 P^T tiles twice on DVE (bf16),
                        # so the cross-partition row-sum matmul streams 4
                        # tiles instead of 16.
                        s8 = [att.tile([P, 512], BF16, tag=f"s8_{i}",
                                       bufs=2, name=f"s8{hh}_{lqc}_{i}")
                              for i in range(8)]
                        for i in range(8):
                            nc.vector.tensor_tensor(s8[i][:], pt[2 * i][:],
                                                    pt[2 * i + 1][:], OP.add)
                        s4 = [att.tile([P, 512], BF16, tag=f"s4_{i}",
                                       bufs=2, name=f"s4{hh}_{lqc}_{i}")
                              for i in range(4)]
                        for i in range(4):
                            nc.vector.tensor_tensor(s4[i][:], s8[2 * i][:],
                                                    s8[2 * i + 1][:], OP.add)
                        # attn @ v first (keeps PE busy while DVE finishes
                        # the pair-adds), then the row-sum matmul.
                        po = [None, None]
                        for dt in range(2):
                            po[dt] = aps.tile([P, 512], F32, tag="po",
                                              bufs=2,
                                              name=f"po{hh}_{lqc}_{dt}")
                            for lk in range(LT):
                                nc.tensor.matmul(
                                    po[dt][:], VV(lk, dt), pt[lk][:],
                                    start=(lk == 0), stop=(lk == LT - 1))
                        # Row sums, pre-broadcast over all 128 partitions
                        # by the all-ones stationary operand.
                        prb = aps.tile([P, 512], F32, tag="prb", bufs=1,
                                       name=f"prb{hh}_{lqc}")
                        for i in range(4):
                            nc.tensor.matmul(prb[:], ones128[:], s4[i][:],
                                             start=(i == 0), stop=(i == 3))
                        rb = att.tile([P, 512], F32, tag="rb", bufs=2,
                                      name=f"rb{hh}_{lqc}")
                        nc.vector.reciprocal_approx_fast(rb[:], prb[:])
                        rb_wu = rb.rearrange("p (u w) -> p w u", w=8)
                        for dt in range(2):
                            # normalize + ColPali gather in one op:
                            # G[:, j*256 + 64*lqc + u] = po[:, 8u+j]*rb[:, 8u+j]
                            g_dst = G[hh][dt].rearrange(
                                "p (w r) -> p w r",
                                w=8)[:, :, 64 * lqc:64 * lqc + 64]
                            nc.vector.tensor_tensor(
                                g_dst,
                                po[dt].rearrange("p (u w) -> p w u", w=8),
                                rb_wu, OP.mult)
                    o_proj(hh)

    nc.compile()
    return nc


_NC = None


def _get_nc():
    global _NC
    if _NC is None:
        _NC = build_program()
    return _NC


def _pack(a):
    """[nb*128, C] -> [128, nb*C], block-major: out[p, i*C+j] = a[i*128+p, j]."""
    R, C = a.shape
    nb = R // P
    return np.ascontiguousarray(
        a.reshape(nb, P, C).transpose(1, 0, 2).reshape(P, nb * C))


def make_in_maps(hidden_states, cos, sin, Wq, Wk, Wv, Wo):
    bf = ml_dtypes.bfloat16
    hs = np.asarray(hidden_states, np.float32)
    cosT = np.asarray(cos, np.float32).T.astype(bf)  # [D, L]
    sinT = np.asarray(sin, np.float32).T.astype(bf)
    xtb = [hs[b].T.astype(bf) for b in range(B)]     # [E, L]
    # xt packed: [128, lc*8192 + e*512 + j]
    xt_p = [np.concatenate(
        [_pack(x[:, lc * 512:(lc + 1) * 512]) for lc in range(4)], axis=1)
        for x in xtb]
    # cs packed: [128, lc*2048 + s*1024 + h*512 + j]
    cs_p = np.concatenate(
        [_pack(src[:, lc * 512:(lc + 1) * 512])
         for lc in range(4) for src in (cosT, sinT)], axis=1)
    wkv_p = np.concatenate(
        [_pack(np.asarray(Wk, np.float32).T.astype(bf)),
         _pack(np.asarray(Wv, np.float32).T.astype(bf))], axis=1)
    wqT = np.asarray(Wq, np.float32).T.astype(bf)    # [E, HD]
    wot_p = _pack(np.asarray(Wo, np.float32).T.astype(bf))
    in_maps = []
    for c in range(N_CORES):
        b, ql = c // 4, c % 4
        sl = slice(ql * 512, (ql + 1) * 512)
        cso_p = np.concatenate(
            [_pack(cosT[:, sl]), _pack(sinT[:, sl])], axis=1)
        in_maps.append({
            "xt": xt_p[b],
            "xto": _pack(xtb[b][:, sl]),
            "cs": cs_p,
            "cso": cso_p,
            "wkv": wkv_p,
            "wqt": _pack(wqT[:, sl]),
            "wot": wot_p,
        })
    return in_maps


def assemble(results):
    y = np.empty((B, L, E), np.float32)
    for c in range(N_CORES):
        b, ql = c // 4, c % 4
        y[b, ql * 512:(ql + 1) * 512, :] = results[c]["out"]
    return y


def kernel(hidden_states, attention_mask, cos, sin, Wq, Wk, Wv, Wo):
    # attention_mask is additive and all-zero per the problem spec; it is
    # accepted for signature compatibility but not shipped to the device.
    nc = _get_nc()
    in_maps = make_in_maps(hidden_states, cos, sin, Wq, Wk, Wv, Wo)
    res = run_bass_kernel_spmd(nc, in_maps, core_ids=list(range(N_CORES)))
    return assemble(res.results)


# revision 7
# speedup vs baseline: 1.1161x; 1.0237x over previous
"""ColPali MQA attention block on 8 Trainium2 NeuronCores.

The reference contains the ColPali reshape quirk: the attention output
[B, H, L, 1, D] is reshaped row-major straight to [B, L, H*D], which mixes
heads and positions.  Output row l' therefore depends ONLY on head
h = l'//256, gathering positions (l'%256)*8 + j for j in 0..7:

    Y[b, l', e] = sum_{j,d} O[b, l'//256, (l'%256)*8+j, d] * Wo[e, j*256+d]

Sharding: core c -> batch b=c//4 and heads {h0, h0+1} with h0=2*(c%4).
Q projection + attention for the core's 2 heads over the full sequence and
o_proj for output rows [512*(c%4), +512) are per-core.  K/V projection is
split across the 4-core batch group: core c computes K/V (with K-RoPE) for
positions [512*(c%4), +512) only, then two AllGathers (k then v, bf16,
256KB/rank each) land full K/V on every core while the Q projection keeps
the PE busy.  Per-core outputs are disjoint [512, 2048] slices of the
[2, 2048, 2048] output -> no further cross-core communication.

All inputs are pre-cast to bf16 AND pre-packed tile-major on the host, so
every device DMA is one fat contiguous transfer (2-16KB per partition
line) straight into its SBUF resident layout -- no staging, no on-device
conversion, no small-line DMA inefficiency.

Layouts (contraction dim always on SBUF partitions; zero on-device
transposes):
  - q, k produced transposed ([D, L]) by making W the stationary operand.
  - v produced natural ([L, D]) by making X the stationary operand.
  - scores computed transposed: S^T[lk, lq] = k @ q_h^T, so the exp output
    P^T[lk, lq] directly feeds O^T[d, lq] = v^T @ P^T as moving operand.
  - softmax row sums over lk (= partitions): the 16 P^T tiles are first
    pair-reduced twice on the vector engine (bf16), then a single all-ones
    [128,128] stationary matmul pass over the 4 partial tiles lands the
    sums pre-broadcast across all 128 partitions; reciprocal_approx_fast
    gives the scale tile.  The normalize multiply writes through a
    (u w)->(w u) access pattern that performs the ColPali gather for free,
    producing G[c, r] = O^T[d, r*8+j] (c = j*256+d) which is directly the
    stationary operand of o_proj.
"""

import numpy as np
import ml_dtypes

import concourse.mybir as mybir
import concourse.tile as tile
from concourse import bacc
from concourse.bass_utils import run_bass_kernel_spmd

F32 = mybir.dt.float32
BF16 = mybir.dt.bfloat16
AF = mybir.ActivationFunctionType
OP = mybir.AluOpType

B, L, H, D, E = 2, 2048, 8, 256, 2048
HD = H * D  # 2048
P = 128
EC = E // P  # 16 e-chunks
LT = L // P  # 16 l-tiles
SCALING = D ** -0.5  # 1/16
N_CORES = 8
RG = [[0, 1, 2, 3], [4, 5, 6, 7]]  # batch groups share K/V


def build_program():
    nc = bacc.Bacc("TRN2", target_bir_lowering=False, debug=False,
                   num_devices=N_CORES)

    # Tile-major packed inputs (see make_in_maps for the host-side layout).
    xt = nc.dram_tensor("xt", [P, 4 * EC * 512], BF16,
                        kind="ExternalInput").ap()
    xto = nc.dram_tensor("xto", [P, EC * 512], BF16,
                         kind="ExternalInput").ap()
    cs = nc.dram_tensor("cs", [P, 4 * 2048], BF16, kind="ExternalInput").ap()
    cso = nc.dram_tensor("cso", [P, 2048], BF16, kind="ExternalInput").ap()
    wkv = nc.dram_tensor("wkv", [P, 2 * EC * D], BF16,
                         kind="ExternalInput").ap()
    wqt = nc.dram_tensor("wqt", [P, EC * 512], BF16,
                         kind="ExternalInput").ap()
    wot = nc.dram_tensor("wot", [P, EC * E], BF16, kind="ExternalInput").ap()
    out = nc.dram_tensor("out", [4 * P, E], F32, kind="ExternalOutput").ap()

    with tile.TileContext(nc) as tc:
        with tc.tile_pool(name="res", bufs=1) as res, \
             tc.tile_pool(name="dram", bufs=1, space="DRAM") as dram:
            # K/V resident layout, one fat tile: rank r owns columns
            # [r*2048, (r+1)*2048) = [kT0(512) | kT1(512) | v0..v3(4x256)].
            kv_sb = res.tile([P, 4 * 2048], BF16, tag="kv", name="kv_sb")

            def KT(i, lk):
                base = (lk // 4) * 2048 + i * 512 + (lk % 4) * P
                return kv_sb[:, base:base + P]

            def VV(lk, dt):
                base = (lk // 4) * 2048 + 1024 + (lk % 4) * D + dt * P
                return kv_sb[:, base:base + P]

            # q^T for the core's two heads: 4 dq-tiles x [128, L]
            qT = [res.tile([P, L], BF16, tag=f"qT{i}", name=f"qT{i}")
                  for i in range(4)]
            # own K/V chunk staging (pre-AllGather)
            kTo = res.tile([P, 1024], BF16, tag="kTo", name="kTo")
            vo = res.tile([P, 1024], BF16, tag="vo", name="vo")
            ones128 = res.tile([P, P], BF16, tag="ones128", name="ones128")
            nc.vector.memset(ones128[:], 1.0)

            # K/V AllGather bounce buffers (HBM).
            in_k = dram.tile([P, 1024], BF16, name="in_k")
            out_k = dram.tile([4 * P, 1024], BF16, name="out_k")
            in_v = dram.tile([P, 1024], BF16, name="in_v")
            out_v = dram.tile([4 * P, 1024], BF16, name="out_v")

            def _rope(p0, p1, out0, out1, cs4, pool, tag):
                cos0, cos1, sin0, sin1 = cs4
                ta = pool.tile([P, 512], F32, tag="ropetmp", bufs=4,
                               name=f"ta{tag}")
                tb = pool.tile([P, 512], F32, tag="ropetmp", bufs=4,
                               name=f"tb{tag}")
                nc.vector.tensor_tensor(ta[:], p0[:], cos0, OP.mult)
                nc.vector.tensor_tensor(tb[:], p1[:], sin0, OP.mult)
                nc.vector.tensor_tensor(out0, ta[:], tb[:], OP.subtract)
                tc2 = pool.tile([P, 512], F32, tag="ropetmp", bufs=4,
                                name=f"tc{tag}")
                td = pool.tile([P, 512], F32, tag="ropetmp", bufs=4,
                               name=f"td{tag}")
                nc.vector.tensor_tensor(tc2[:], p1[:], cos1, OP.mult)
                nc.vector.tensor_tensor(td[:], p0[:], sin1, OP.mult)
                nc.vector.tensor_tensor(out1, tc2[:], td[:], OP.add)

            # ---------------- Phase 1: projections + RoPE ----------------
            with tc.tile_pool(name="proj", bufs=1) as proj, \
                 tc.tile_pool(name="proj_ps", space="PSUM", bufs=1) as pps:
                wkv_sb = proj.tile([P, 2 * EC * D], BF16, tag="wkv",
                                   name="wkv_sb")
                xto_sb = proj.tile([P, EC * 512], BF16, tag="xto",
                                   name="xto_sb")
                cso_sb = proj.tile([P, 2048], BF16, tag="cso", name="cso_sb")
                wqt_sb = proj.tile([P, EC * 512], BF16, tag="wqt",
                                   name="wqt_sb")
                xt_sb = proj.tile([P, 4 * EC * 512], BF16, tag="xt",
                                  name="xt_sb")
                cs_sb = proj.tile([P, 4 * 2048], BF16, tag="cs",
                                  name="cs_sb")

                def WK(e, i):
                    return wkv_sb[:, e * D + i * P:e * D + (i + 1) * P]

                def WV(e):
                    return wkv_sb[:, EC * D + e * D:EC * D + (e + 1) * D]

                def XTO(e):
                    return xto_sb[:, e * 512:(e + 1) * 512]

                def XS(lc, e):
                    base = lc * EC * 512 + e * 512
                    return xt_sb[:, base:base + 512]

                def WQ(e, j):
                    return wqt_sb[:, e * 512 + j * P:e * 512 + (j + 1) * P]

                def CS(lc, s, h):
                    base = lc * 2048 + s * 1024 + h * 512
                    return cs_sb[:, base:base + 512]

                # --- phase-1a DMAs: own-chunk K/V inputs.  Chunked so the
                # K projection starts as soon as the first blocks land and
                # the AllGathers trigger as early as possible. ---
                nc.sync.dma_start(out=wkv_sb[:, 0:EC * D],
                                  in_=wkv[:, 0:EC * D])
                for cch in range(4):
                    csl = slice(cch * 4 * 512, (cch + 1) * 4 * 512)
                    nc.sync.dma_start(out=xto_sb[:, csl], in_=xto[:, csl])
                nc.sync.dma_start(out=cso_sb[:], in_=cso[:, :])
                nc.sync.dma_start(out=wkv_sb[:, EC * D:2 * EC * D],
                                  in_=wkv[:, EC * D:2 * EC * D])
                nc.sync.dma_start(out=wqt_sb[:], in_=wqt[:, :])

                # K projection (own 512 positions) + RoPE.
                pk0 = pps.tile([P, 512], F32, tag="p1a", bufs=3, name="pk0")
                pk1 = pps.tile([P, 512], F32, tag="p1a", bufs=3, name="pk1")
                for e in range(EC):
                    st, sp = (e == 0), (e == EC - 1)
                    nc.tensor.matmul(pk0[:], WK(e, 0), XTO(e),
                                     start=st, stop=sp)
                    nc.tensor.matmul(pk1[:], WK(e, 1), XTO(e),
                                     start=st, stop=sp)
                _rope(pk0, pk1, kTo[:, 0:512], kTo[:, 512:1024],
                      (cso_sb[:, 0:512], cso_sb[:, 512:1024],
                       cso_sb[:, 1024:1536], cso_sb[:, 1536:2048]),
                      proj, "k")
                nc.gpsimd.dma_start(out=in_k[:], in_=kTo[:])
                nc.gpsimd.collective_compute(
                    "AllGather", mybir.AluOpType.bypass, replica_groups=RG,
                    ins=[in_k.opt()], outs=[out_k.opt()])

                # V projection (own 512 positions).
                for lt in range(4):
                    pv = pps.tile([P, D], F32, tag="p1a", bufs=3,
                                  name=f"pv{lt}")
                    for e in range(EC):
                        nc.tensor.matmul(pv[:],
                                         XTO(e)[:, lt * P:(lt + 1) * P],
                                         WV(e),
                                         start=(e == 0), stop=(e == EC - 1))
                    nc.vector.tensor_copy(vo[:, lt * D:(lt + 1) * D], pv[:])
                nc.gpsimd.dma_start(out=in_v[:], in_=vo[:])
                nc.gpsimd.collective_compute(
                    "AllGather", mybir.AluOpType.bypass, replica_groups=RG,
                    ins=[in_v.opt()], outs=[out_v.opt()])
                # Scatter the gathered shards into the resident K/V layout.
                for r in range(4):
                    nc.gpsimd.dma_start(
                        out=kv_sb[:, r * 2048:r * 2048 + 1024],
                        in_=out_k[r * P:(r + 1) * P, :])
                for r in range(4):
                    nc.gpsimd.dma_start(
                        out=kv_sb[:, r * 2048 + 1024:r * 2048 + 2048],
                        in_=out_v[r * P:(r + 1) * P, :])

                # --- Phase 1b: Q projection + RoPE over the full sequence,
                # overlapping the AllGathers ---
                for lc in range(4):
                    sl = slice(lc * 512, (lc + 1) * 512)
                    nc.sync.dma_start(
                        out=xt_sb[:, lc * EC * 512:(lc + 1) * EC * 512],
                        in_=xt[:, lc * EC * 512:(lc + 1) * EC * 512])
                    nc.sync.dma_start(
                        out=cs_sb[:, lc * 2048:(lc + 1) * 2048],
                        in_=cs[:, lc * 2048:(lc + 1) * 2048])

                    pq = [pps.tile([P, 512], F32, tag="pq", bufs=5,
                                   name=f"pq{lc}_{j}") for j in range(4)]
                    for e in range(EC):
                        st, sp = (e == 0), (e == EC - 1)
                        xs = XS(lc, e)
                        for j in range(4):
                            nc.tensor.matmul(pq[j][:], WQ(e, j), xs,
                                             start=st, stop=sp)
                    _rope(pq[0], pq[1], qT[0][:, sl], qT[1][:, sl],
                          (CS(lc, 0, 0), CS(lc, 0, 1),
                           CS(lc, 1, 0), CS(lc, 1, 1)), proj, f"q0{lc}")
                    _rope(pq[2], pq[3], qT[2][:, sl], qT[3][:, sl],
                          (CS(lc, 0, 0), CS(lc, 0, 1),
                           CS(lc, 1, 0), CS(lc, 1, 1)), proj, f"q1{lc}")

            # ------------- Phase 2: attention + o_proj -------------
            with tc.tile_pool(name="att", bufs=1) as att, \
                 tc.tile_pool(name="att_ps", space="PSUM", bufs=1) as aps:
                # G[hh][half]: gathered, normalized O^T.  G[c-row, col] with
                # c-row = d within half, column layout j*256 + r.
                G = [[att.tile([P, L], BF16, tag=f"G{hh}{dt}",
                               name=f"G{hh}{dt}") for dt in range(2)]
                     for hh in range(2)]
                wot_sb = att.tile([P, EC * E], BF16, tag="wot",
                                  name="wot_sb")
                for i in range(4):
                    qe = EC * E // 4
                    nc.sync.dma_start(out=wot_sb[:, i * qe:(i + 1) * qe],
                                      in_=wot[:, i * qe:(i + 1) * qe])

                def WOT(m, eg):
                    base = m * E + eg * 512
                    return wot_sb[:, base:base + 512]

                def o_proj(a_idx):
                    for rh in range(2):
                        rt = a_idx * 2 + rh
                        for eg in range(4):
                            py = aps.tile([P, 512], F32, tag="py", bufs=2,
                                          name=f"py{rt}_{eg}")
                            for m in range(EC):
                                lhsT = G[a_idx][m % 2][
                                    :, (m // 2) * 256 + rh * P:
                                       (m // 2) * 256 + rh * P + P]
                                nc.tensor.matmul(py[:], lhsT, WOT(m, eg),
                                                 start=(m == 0),
                                                 stop=(m == EC - 1))
                            ysb = att.tile([P, 512], F32, tag="ysb", bufs=3,
                                           name=f"ysb{rt}_{eg}")
                            nc.scalar.copy(ysb[:], py[:])
                            nc.sync.dma_start(
                                out=out[rt * P:(rt + 1) * P,
                                        eg * 512:(eg + 1) * 512],
                                in_=ysb[:])

                for hh in range(2):
                    qh0, qh1 = qT[2 * hh], qT[2 * hh + 1]
                    for lqc in range(4):
                        qsl = slice(lqc * 512, (lqc + 1) * 512)
                        pt = [att.tile([P, 512], BF16, tag=f"pt{i}", bufs=2,
                                       name=f"pt{hh}_{lqc}_{i}")
                              for i in range(LT)]
                        for lk in range(LT):
                            ps = aps.tile([P, 512], F32, tag="ps", bufs=3,
                                          name=f"ps{hh}_{lqc}_{lk}")
                            nc.tensor.matmul(ps[:], KT(0, lk), qh0[:, qsl],
                                             start=True, stop=False)
                            nc.tensor.matmul(ps[:], KT(1, lk), qh1[:, qsl],
                                             start=False, stop=True)
                            nc.scalar.activation(pt[lk][:], ps[:], AF.Exp,
                                                 scale=float(SCALING))
                        # Pair-reduce the 16 P^T tiles twice on DVE (bf16),
                        # so the cross-partition row-sum matmul streams 4
                        # tiles instead of 16.
                        s8 = [att.tile([P, 512], BF16, tag=f"s8_{i}",
                                       bufs=2, name=f"s8{hh}_{lqc}_{i}")
                              for i in range(8)]
                        for i in range(8):
                            nc.vector.tensor_tensor(s8[i][:], pt[2 * i][:],
                                                    pt[2 * i + 1][:], OP.add)
                        s4 = [att.tile([P, 512], BF16, tag=f"s4_{i}",
                                       bufs=2, name=f"s4{hh}_{lqc}_{i}")
                              for i in range(4)]
                        for i in range(4):
                            nc.vector.tensor_tensor(s4[i][:], s8[2 * i][:],
                                                    s8[2 * i + 1][:], OP.add)
                        # attn @ v first (keeps PE busy while DVE finishes
                        # the pair-adds), then the row-sum matmul.
                        po = [None, None]
                        for dt in range(2):
                            po[dt] = aps.tile([P, 512], F32, tag="po",
                                              bufs=2,
                                              name=f"po{hh}_{lqc}_{dt}")
                            for lk in range(LT):
                                nc.tensor.matmul(
                                    po[dt][:], VV(lk, dt), pt[lk][:],
                                    start=(lk == 0), stop=(lk == LT - 1))
                        # Row sums, pre-broadcast over all 128 partitions
                        # by the all-ones stationary operand.
                        prb = aps.tile([P, 512], F32, tag="prb", bufs=1,
                                       name=f"prb{hh}_{lqc}")
                        for i in range(4):
                            nc.tensor.matmul(prb[:], ones128[:], s4[i][:],
                                             start=(i == 0), stop=(i == 3))
                        rb = att.tile([P, 512], F32, tag="rb", bufs=2,
                                      name=f"rb{hh}_{lqc}")
                        nc.vector.reciprocal_approx_fast(rb[:], prb[:])
                        rb_wu = rb.rearrange("p (u w) -> p w u", w=8)
                        for dt in range(2):
                            # normalize + ColPali gather in one op:
                            # G[:, j*256 + 64*lqc + u] = po[:, 8u+j]*rb[:, 8u+j]
                            g_dst = G[hh][dt].rearrange(
                                "p (w r) -> p w r",
                                w=8)[:, :, 64 * lqc:64 * lqc + 64]
                            nc.vector.tensor_tensor(
                                g_dst,
                                po[dt].rearrange("p (u w) -> p w u", w=8),
                                rb_wu, OP.mult)
                    o_proj(hh)

    nc.compile()
    return nc


_NC = None


def _get_nc():
    global _NC
    if _NC is None:
        _NC = build_program()
    return _NC


def _pack(a):
    """[nb*128, C] -> [128, nb*C], block-major: out[p, i*C+j] = a[i*128+p, j]."""
    R, C = a.shape
    nb = R // P
    return np.ascontiguousarray(
        a.reshape(nb, P, C).transpose(1, 0, 2).reshape(P, nb * C))


def make_in_maps(hidden_states, cos, sin, Wq, Wk, Wv, Wo):
    bf = ml_dtypes.bfloat16
    hs = np.asarray(hidden_states, np.float32)
    cosT = np.asarray(cos, np.float32).T.astype(bf)  # [D, L]
    sinT = np.asarray(sin, np.float32).T.astype(bf)
    xtb = [hs[b].T.astype(bf) for b in range(B)]     # [E, L]
    # xt packed: [128, lc*8192 + e*512 + j]
    xt_p = [np.concatenate(
        [_pack(x[:, lc * 512:(lc + 1) * 512]) for lc in range(4)], axis=1)
        for x in xtb]
    # cs packed: [128, lc*2048 + s*1024 + h*512 + j]
    cs_p = np.concatenate(
        [_pack(src[:, lc * 512:(lc + 1) * 512])
         for lc in range(4) for src in (cosT, sinT)], axis=1)
    wkv_p = np.concatenate(
        [_pack(np.asarray(Wk, np.float32).T.astype(bf)),
         _pack(np.asarray(Wv, np.float32).T.astype(bf))], axis=1)
    wqT = np.asarray(Wq, np.float32).T.astype(bf)    # [E, HD]
    wot_p = _pack(np.asarray(Wo, np.float32).T.astype(bf))
    in_maps = []
    for c in range(N_CORES):
        b, ql = c // 4, c % 4
        sl = slice(ql * 512, (ql + 1) * 512)
        cso_p = np.concatenate(
            [_pack(cosT[:, sl]), _pack(sinT[:, sl])], axis=1)
        in_maps.append({
            "xt": xt_p[b],
            "xto": _pack(xtb[b][:, sl]),
            "cs": cs_p,
            "cso": cso_p,
            "wkv": wkv_p,
            "wqt": _pack(wqT[:, sl]),
            "wot": wot_p,
        })
    return in_maps


def assemble(results):
    y = np.empty((B, L, E), np.float32)
    for c in range(N_CORES):
        b, ql = c // 4, c % 4
        y[b, ql * 512:(ql + 1) * 512, :] = results[c]["out"]
    return y


def kernel(hidden_states, attention_mask, cos, sin, Wq, Wk, Wv, Wo):
    # attention_mask is additive and all-zero per the problem spec; it is
    # accepted for signature compatibility but not shipped to the device.
    nc = _get_nc()
    in_maps = make_in_maps(hidden_states, cos, sin, Wq, Wk, Wv, Wo)
    res = run_bass_kernel_spmd(nc, in_maps, core_ids=list(range(N_CORES)))
    return assemble(res.results)


# revision 17
# speedup vs baseline: 1.1542x; 1.0341x over previous
"""ColPali MQA attention block on 8 Trainium2 NeuronCores.

The reference contains the ColPali reshape quirk: the attention output
[B, H, L, 1, D] is reshaped row-major straight to [B, L, H*D], which mixes
heads and positions.  Output row l' therefore depends ONLY on head
h = l'//256, gathering positions (l'%256)*8 + j for j in 0..7:

    Y[b, l', e] = sum_{j,d} O[b, l'//256, (l'%256)*8+j, d] * Wo[e, j*256+d]

Sharding: core c -> batch b=c//4 and heads {h0, h0+1} with h0=2*(c%4).
Each core computes K/V projection for its batch (replicated inside the
4-core batch group), Q projection + attention for its 2 heads over the full
sequence, and o_proj for output rows [512*(c%4), +512).  Per-core outputs
are disjoint [512, 2048] slices of the [2, 2048, 2048] output -> no
cross-core communication.  (A K/V AllGather variant was measured: it saves
~50us of PE work but its SDMA burst trips the chip's power throttler
(K=13/16 for the next ~200us firmware period), costing more than it saves.)

All inputs are pre-cast to bf16 AND pre-packed tile-major on the host, so
every device DMA is one fat contiguous transfer (2-16KB per partition
line) straight into its SBUF-resident layout -- no staging, no on-device
conversion, no small-line DMA inefficiency.

Layouts (contraction dim always on SBUF partitions; zero on-device
transposes):
  - q, k produced transposed ([D, L]) by making W the stationary operand.
  - v produced natural ([L, D]) by making X the stationary operand.
  - K/V/Q projections interleaved per 512-column block of x^T so the PE
    ramps as blocks arrive and psum WAR hazards never stall.
  - scores computed transposed: S^T[lk, lq] = k @ q_h^T, so the exp output
    P^T[lk, lq] directly feeds O^T[d, lq] = v^T @ P^T as moving operand.
  - softmax row sums over lk (= partitions): the 16 P^T tiles are first
    pair-reduced twice on the vector engine (bf16), then a single all-ones
    [128,128] stationary matmul pass over the 4 partial tiles lands the
    sums pre-broadcast across all 128 partitions; reciprocal_approx_fast
    gives the scale tile.  The normalize multiply writes through a
    (u w)->(w u) access pattern that performs the ColPali gather for free,
    producing G[c, r] = O^T[d, r*8+j] (c = j*256+d) which is directly the
    stationary operand of o_proj.
"""

import numpy as np
import ml_dtypes

import concourse.mybir as mybir
import concourse.tile as tile
from concourse import bacc
from concourse.bass_utils import run_bass_kernel_spmd

F32 = mybir.dt.float32
BF16 = mybir.dt.bfloat16
AF = mybir.ActivationFunctionType
OP = mybir.AluOpType

B, L, H, D, E = 2, 2048, 8, 256, 2048
HD = H * D  # 2048
P = 128
EC = E // P  # 16 e-chunks
LT = L // P  # 16 l-tiles
SCALING = D ** -0.5  # 1/16
N_CORES = 8


def build_program():
    nc = bacc.Bacc("TRN2", target_bir_lowering=False, debug=False,
                   num_devices=N_CORES)

    # Tile-major packed inputs (see make_in_maps for the host-side layout).
    xt = nc.dram_tensor("xt", [P, 4 * EC * 512], BF16,
                        kind="ExternalInput").ap()
    cs = nc.dram_tensor("cs", [P, 4 * 2048], BF16, kind="ExternalInput").ap()
    wkv = nc.dram_tensor("wkv", [P, 2 * EC * D], BF16,
                         kind="ExternalInput").ap()
    wqt = nc.dram_tensor("wqt", [P, EC * 512], BF16,
                         kind="ExternalInput").ap()
    wot = nc.dram_tensor("wot", [P, EC * E], BF16, kind="ExternalInput").ap()
    out = nc.dram_tensor("out", [4 * P, E], F32, kind="ExternalOutput").ap()

    with tile.TileContext(nc) as tc:
        with tc.tile_pool(name="res", bufs=1) as res:
            kT = [res.tile([P, L], BF16, tag=f"kT{i}", name=f"kT{i}")
                  for i in range(2)]
            v_bf = [res.tile([P, D], BF16, tag=f"v{i}", name=f"v{i}")
                    for i in range(LT)]
            # q^T for the core's two heads: 4 dq-tiles x [128, L]
            qT = [res.tile([P, L], BF16, tag=f"qT{i}", name=f"qT{i}")
                  for i in range(4)]
            ones128 = res.tile([P, P], BF16, tag="ones128", name="ones128")
            nc.vector.memset(ones128[:], 1.0)

            def _rope(p0, p1, out0, out1, cs4, pool, tag):
                cos0, cos1, sin0, sin1 = cs4
                ta = pool.tile([P, 512], F32, tag="ropetmp", bufs=4,
                               name=f"ta{tag}")
                tb = pool.tile([P, 512], F32, tag="ropetmp", bufs=4,
                               name=f"tb{tag}")
                nc.vector.tensor_tensor(ta[:], p0[:], cos0, OP.mult)
                nc.vector.tensor_tensor(tb[:], p1[:], sin0, OP.mult)
                nc.vector.tensor_tensor(out0, ta[:], tb[:], OP.subtract)
                tc2 = pool.tile([P, 512], F32, tag="ropetmp", bufs=4,
                                name=f"tc{tag}")
                td = pool.tile([P, 512], F32, tag="ropetmp", bufs=4,
                               name=f"td{tag}")
                nc.vector.tensor_tensor(tc2[:], p1[:], cos1, OP.mult)
                nc.vector.tensor_tensor(td[:], p0[:], sin1, OP.mult)
                nc.vector.tensor_tensor(out1, tc2[:], td[:], OP.add)

            # ---------------- Phase 1: projections + RoPE ----------------
            with tc.tile_pool(name="proj", bufs=1) as proj, \
                 tc.tile_pool(name="proj_ps", space="PSUM", bufs=1) as pps:
                wkv_sb = proj.tile([P, 2 * EC * D], BF16, tag="wkv",
                                   name="wkv_sb")
                wqt_sb = proj.tile([P, EC * 512], BF16, tag="wqt",
                                   name="wqt_sb")
                xt_sb = proj.tile([P, 4 * EC * 512], BF16, tag="xt",
                                  name="xt_sb")
                cs_sb = proj.tile([P, 4 * 2048], BF16, tag="cs",
                                  name="cs_sb")

                def WK(e, i):
                    return wkv_sb[:, e * D + i * P:e * D + (i + 1) * P]

                def WV(e):
                    return wkv_sb[:, EC * D + e * D:EC * D + (e + 1) * D]

                def XS(lc, e):
                    base = lc * EC * 512 + e * 512
                    return xt_sb[:, base:base + 512]

                def WQ(e, j):
                    return wqt_sb[:, e * 512 + j * P:e * 512 + (j + 1) * P]

                def CS(lc, s, h):
                    base = lc * 2048 + s * 1024 + h * 512
                    return cs_sb[:, base:base + 512]

                # DMA issue order tracks PE consumption: wk/wv, then per-lc
                # x^T + cos/sin blocks; wqt after the first x block.
                nc.sync.dma_start(out=wkv_sb[:], in_=wkv[:, :])
                for half in range(2):
                    hs_ = slice(half * EC * 256, (half + 1) * EC * 256)
                    nc.sync.dma_start(out=xt_sb[:, hs_], in_=xt[:, hs_])
                nc.sync.dma_start(out=cs_sb[:, 0:2048], in_=cs[:, 0:2048])
                nc.sync.dma_start(out=wqt_sb[:], in_=wqt[:, :])
                for lc in range(1, 4):
                    lsl = slice(lc * EC * 512, (lc + 1) * EC * 512)
                    nc.sync.dma_start(out=xt_sb[:, lsl], in_=xt[:, lsl])
                    nc.sync.dma_start(
                        out=cs_sb[:, lc * 2048:(lc + 1) * 2048],
                        in_=cs[:, lc * 2048:(lc + 1) * 2048])

                # Per 512-column block: K proj + RoPE, V proj, Q proj +
                # RoPE.  PE streams continuously; each block's inputs are
                # one fat DMA ahead.
                for lc in range(4):
                    sl = slice(lc * 512, (lc + 1) * 512)
                    cs4 = (CS(lc, 0, 0), CS(lc, 0, 1),
                           CS(lc, 1, 0), CS(lc, 1, 1))

                    pk0 = pps.tile([P, 512], F32, tag="pk", bufs=2,
                                   name=f"pk0_{lc}")
                    pk1 = pps.tile([P, 512], F32, tag="pk", bufs=2,
                                   name=f"pk1_{lc}")
                    for e in range(EC):
                        st, sp = (e == 0), (e == EC - 1)
                        xs = XS(lc, e)
                        nc.tensor.matmul(pk0[:], WK(e, 0), xs,
                                         start=st, stop=sp)
                        nc.tensor.matmul(pk1[:], WK(e, 1), xs,
                                         start=st, stop=sp)
                    _rope(pk0, pk1, kT[0][:, sl], kT[1][:, sl], cs4,
                          proj, f"k{lc}")

                    for lt in range(4 * lc, 4 * lc + 4):
                        pv = pps.tile([P, D], F32, tag="pv", bufs=2,
                                      name=f"pv{lt}")
                        for e in range(EC):
                            nc.tensor.matmul(
                                pv[:],
                                XS(lc, e)[:, (lt % 4) * P:(lt % 4 + 1) * P],
                                WV(e),
                                start=(e == 0), stop=(e == EC - 1))
                        nc.vector.tensor_copy(v_bf[lt][:], pv[:])

                    pq = [pps.tile([P, 512], F32, tag="pq", bufs=4,
                                   name=f"pq{lc}_{j}") for j in range(4)]
                    for e in range(EC):
                        st, sp = (e == 0), (e == EC - 1)
                        xs = XS(lc, e)
                        for j in range(4):
                            nc.tensor.matmul(pq[j][:], WQ(e, j), xs,
                                             start=st, stop=sp)
                    _rope(pq[0], pq[1], qT[0][:, sl], qT[1][:, sl], cs4,
                          proj, f"q0{lc}")
                    _rope(pq[2], pq[3], qT[2][:, sl], qT[3][:, sl], cs4,
                          proj, f"q1{lc}")

            # ------------- Phase 2: attention + o_proj -------------
            with tc.tile_pool(name="att", bufs=1) as att, \
                 tc.tile_pool(name="att_ps", space="PSUM", bufs=1) as aps:
                # G[hh][half]: gathered, normalized O^T.  G[c-row, col] with
                # c-row = d within half, column layout j*256 + r.
                G = [[att.tile([P, L], BF16, tag=f"G{hh}{dt}",
                               name=f"G{hh}{dt}") for dt in range(2)]
                     for hh in range(2)]
                wot_sb = att.tile([P, EC * E], BF16, tag="wot",
                                  name="wot_sb")
                for i in range(4):
                    qe = EC * E // 4
                    nc.sync.dma_start(out=wot_sb[:, i * qe:(i + 1) * qe],
                                      in_=wot[:, i * qe:(i + 1) * qe])

                def WOT(m, eg):
                    base = m * E + eg * 512
                    return wot_sb[:, base:base + 512]

                def o_proj(a_idx):
                    for rh in range(2):
                        rt = a_idx * 2 + rh
                        for eg in range(4):
                            py = aps.tile([P, 512], F32, tag="py", bufs=2,
                                          name=f"py{rt}_{eg}")
                            for m in range(EC):
                                lhsT = G[a_idx][m % 2][
                                    :, (m // 2) * 256 + rh * P:
                                       (m // 2) * 256 + rh * P + P]
                                nc.tensor.matmul(py[:], lhsT, WOT(m, eg),
                                                 start=(m == 0),
                                                 stop=(m == EC - 1))
                            ysb = att.tile([P, 512], F32, tag="ysb", bufs=3,
                                           name=f"ysb{rt}_{eg}")
                            nc.scalar.copy(ysb[:], py[:])
                            nc.sync.dma_start(
                                out=out[rt * P:(rt + 1) * P,
                                        eg * 512:(eg + 1) * 512],
                                in_=ysb[:])

                for hh in range(2):
                    qh0, qh1 = qT[2 * hh], qT[2 * hh + 1]
                    for lqc in range(4):
                        qsl = slice(lqc * 512, (lqc + 1) * 512)
                        pt = [att.tile([P, 512], BF16, tag=f"pt{i}", bufs=2,
                                       name=f"pt{hh}_{lqc}_{i}")
                              for i in range(LT)]
                        for lk in range(LT):
                            ps = aps.tile([P, 512], F32, tag="ps", bufs=3,
                                          name=f"ps{hh}_{lqc}_{lk}")
                            nc.tensor.matmul(ps[:],
                                             kT[0][:, lk * P:(lk + 1) * P],
                                             qh0[:, qsl],
                                             start=True, stop=False)
                            nc.tensor.matmul(ps[:],
                                             kT[1][:, lk * P:(lk + 1) * P],
                                             qh1[:, qsl],
                                             start=False, stop=True)
                            nc.scalar.activation(pt[lk][:], ps[:], AF.Exp,
                                                 scale=float(SCALING))
                        # Pair-reduce the 16 P^T tiles twice on DVE (bf16),
                        # so the cross-partition row-sum matmul streams 4
                        # tiles instead of 16.
                        s8 = [att.tile([P, 512], BF16, tag=f"s8_{i}",
                                       bufs=2, name=f"s8{hh}_{lqc}_{i}")
                              for i in range(8)]
                        for i in range(8):
                            nc.vector.tensor_tensor(s8[i][:], pt[2 * i][:],
                                                    pt[2 * i + 1][:], OP.add)
                        s4 = [att.tile([P, 512], BF16, tag=f"s4_{i}",
                                       bufs=2, name=f"s4{hh}_{lqc}_{i}")
                              for i in range(4)]
                        for i in range(4):
                            nc.vector.tensor_tensor(s4[i][:], s8[2 * i][:],
                                                    s8[2 * i + 1][:], OP.add)
                        # attn @ v first (keeps PE busy while DVE finishes
                        # the pair-adds), then the row-sum matmul.
                        po = [None, None]
                        for dt in range(2):
                            po[dt] = aps.tile([P, 512], F32, tag="po",
                                              bufs=2,
                                              name=f"po{hh}_{lqc}_{dt}")
                            for lk in range(LT):
                                nc.tensor.matmul(
                                    po[dt][:],
                                    v_bf[lk][:, dt * P:(dt + 1) * P],
                                    pt[lk][:],
                                    start=(lk == 0), stop=(lk == LT - 1))
                        # Row sums, pre-broadcast over all 128 partitions
                        # by the all-ones stationary operand.
                        prb = aps.tile([P, 512], F32, tag="prb", bufs=1,
                                       name=f"prb{hh}_{lqc}")
                        for i in range(4):
                            nc.tensor.matmul(prb[:], ones128[:], s4[i][:],
                                             start=(i == 0), stop=(i == 3))
                        rb = att.tile([P, 512], F32, tag="rb", bufs=2,
                                      name=f"rb{hh}_{lqc}")
                        nc.vector.reciprocal_approx_fast(rb[:], prb[:])
                        rb_wu = rb.rearrange("p (u w) -> p w u", w=8)
                        for dt in range(2):
                            # normalize + ColPali gather in one op:
                            # G[:, j*256 + 64*lqc + u] = po[:, 8u+j]*rb[:, 8u+j]
                            g_dst = G[hh][dt].rearrange(
                                "p (w r) -> p w r",
                                w=8)[:, :, 64 * lqc:64 * lqc + 64]
                            nc.vector.tensor_tensor(
                                g_dst,
                                po[dt].rearrange("p (u w) -> p w u", w=8),
                                rb_wu, OP.mult)
                    o_proj(hh)

    nc.compile()
    return nc


_NC = None


def _get_nc():
    global _NC
    if _NC is None:
        _NC = build_program()
    return _NC


def _pack(a):
    """[nb*128, C] -> [128, nb*C], block-major: out[p, i*C+j] = a[i*128+p, j]."""
    R, C = a.shape
    nb = R // P
    return np.ascontiguousarray(
        a.reshape(nb, P, C).transpose(1, 0, 2).reshape(P, nb * C))


def make_in_maps(hidden_states, cos, sin, Wq, Wk, Wv, Wo):
    bf = ml_dtypes.bfloat16
    hs = np.asarray(hidden_states, np.float32)
    cosT = np.asarray(cos, np.float32).T.astype(bf)  # [D, L]
    sinT = np.asarray(sin, np.float32).T.astype(bf)
    xtb = [hs[b].T.astype(bf) for b in range(B)]     # [E, L]
    # xt packed: [128, lc*8192 + e*512 + j]
    xt_p = [np.concatenate(
        [_pack(x[:, lc * 512:(lc + 1) * 512]) for lc in range(4)], axis=1)
        for x in xtb]
    # cs packed: [128, lc*2048 + s*1024 + h*512 + j]
    cs_p = np.concatenate(
        [_pack(src[:, lc * 512:(lc + 1) * 512])
         for lc in range(4) for src in (cosT, sinT)], axis=1)
    wkv_p = np.concatenate(
        [_pack(np.asarray(Wk, np.float32).T.astype(bf)),
         _pack(np.asarray(Wv, np.float32).T.astype(bf))], axis=1)
    wqT = np.asarray(Wq, np.float32).T.astype(bf)    # [E, HD]
    wot_p = _pack(np.asarray(Wo, np.float32).T.astype(bf))
    in_maps = []
    for c in range(N_CORES):
        b, ql = c // 4, c % 4
        sl = slice(ql * 512, (ql + 1) * 512)
        in_maps.append({
            "xt": xt_p[b],
            "cs": cs_p,
            "wkv": wkv_p,
            "wqt": _pack(np.ascontiguousarray(wqT[:, sl])),
            "wot": wot_p,
        })
    return in_maps


def assemble(results):
    y = np.empty((B, L, E), np.float32)
    for c in range(N_CORES):
        b, ql = c // 4, c % 4
        y[b, ql * 512:(ql + 1) * 512, :] = results[c]["out"]
    return y


def kernel(hidden_states, attention_mask, cos, sin, Wq, Wk, Wv, Wo):
    # attention_mask is additive and all-zero per the problem spec; it is
    # accepted for signature compatibility but not shipped to the device.
    nc = _get_nc()
    in_maps = make_in_maps(hidden_states, cos, sin, Wq, Wk, Wv, Wo)
    res = run_bass_kernel_spmd(nc, in_maps, core_ids=list(range(N_CORES)))
    return assemble(res.results)


# revision 19
# speedup vs baseline: 1.1839x; 1.0257x over previous
"""ColPali MQA attention block on 8 Trainium2 NeuronCores.

The reference contains the ColPali reshape quirk: the attention output
[B, H, L, 1, D] is reshaped row-major straight to [B, L, H*D], which mixes
heads and positions.  Output row l' therefore depends ONLY on head
h = l'//256, gathering positions (l'%256)*8 + j for j in 0..7:

    Y[b, l', e] = sum_{j,d} O[b, l'//256, (l'%256)*8+j, d] * Wo[e, j*256+d]

Sharding: core c -> batch b=c//4 and heads {h0, h0+1} with h0=2*(c%4).
Each core computes K/V projection for its batch (replicated inside the
4-core batch group), Q projection + attention for its 2 heads over the full
sequence, and o_proj for output rows [512*(c%4), +512).  Per-core outputs
are disjoint [512, 2048] slices of the [2, 2048, 2048] output -> no
cross-core communication.  (A K/V AllGather variant was measured: it saves
~50us of PE work but its SDMA burst trips the chip's power throttler
(K=13/16 for the next ~200us firmware period), costing more than it saves.)

All inputs are pre-cast to bf16 AND pre-packed tile-major on the host, so
every device DMA is one fat contiguous transfer (2-16KB per partition
line) straight into its SBUF-resident layout -- no staging, no on-device
conversion, no small-line DMA inefficiency.

Layouts (contraction dim always on SBUF partitions; zero on-device
transposes):
  - q, k produced transposed ([D, L]) by making W the stationary operand.
  - v produced natural ([L, D]) by making X the stationary operand.
  - K/V/Q projections interleaved per 512-column block of x^T so the PE
    ramps as blocks arrive and psum WAR hazards never stall.
  - scores computed transposed: S^T[lk, lq] = k @ q_h^T, so the exp output
    P^T[lk, lq] directly feeds O^T[d, lq] = v^T @ P^T as moving operand.
  - softmax row sums over lk (= partitions): the 16 P^T tiles are first
    pair-reduced twice on the vector engine (bf16), then a single all-ones
    [128,128] stationary matmul pass over the 4 partial tiles lands the
    sums pre-broadcast across all 128 partitions; reciprocal_approx_fast
    gives the scale tile.  The normalize multiply writes through a
    (u w)->(w u) access pattern that performs the ColPali gather for free,
    producing G[c, r] = O^T[d, r*8+j] (c = j*256+d) which is directly the
    stationary operand of o_proj.
"""

import numpy as np
import ml_dtypes

import concourse.mybir as mybir
import concourse.tile as tile
from concourse import bacc
from concourse.bass_utils import run_bass_kernel_spmd

F32 = mybir.dt.float32
BF16 = mybir.dt.bfloat16
AF = mybir.ActivationFunctionType
OP = mybir.AluOpType

B, L, H, D, E = 2, 2048, 8, 256, 2048
HD = H * D  # 2048
P = 128
EC = E // P  # 16 e-chunks
LT = L // P  # 16 l-tiles
SCALING = D ** -0.5  # 1/16
N_CORES = 8


def build_program():
    nc = bacc.Bacc("TRN2", target_bir_lowering=False, debug=False,
                   num_devices=N_CORES)

    # Tile-major packed inputs (see make_in_maps for the host-side layout).
    xt = nc.dram_tensor("xt", [P, 4 * EC * 512], BF16,
                        kind="ExternalInput").ap()
    cs = nc.dram_tensor("cs", [P, 4 * 2048], BF16, kind="ExternalInput").ap()
    wkv = nc.dram_tensor("wkv", [P, 2 * EC * D], BF16,
                         kind="ExternalInput").ap()
    wqt = nc.dram_tensor("wqt", [P, EC * 512], BF16,
                         kind="ExternalInput").ap()
    wot = nc.dram_tensor("wot", [P, EC * E], BF16, kind="ExternalInput").ap()
    out = nc.dram_tensor("out", [4 * P, E], F32, kind="ExternalOutput").ap()

    with tile.TileContext(nc) as tc:
        with tc.tile_pool(name="res", bufs=1) as res:
            kT = [res.tile([P, L], BF16, tag=f"kT{i}", name=f"kT{i}")
                  for i in range(2)]
            v_bf = [res.tile([P, D], BF16, tag=f"v{i}", name=f"v{i}")
                    for i in range(LT)]
            # q^T for the core's two heads: 4 dq-tiles x [128, L]
            qT = [res.tile([P, L], BF16, tag=f"qT{i}", name=f"qT{i}")
                  for i in range(4)]
            ones128 = res.tile([P, P], BF16, tag="ones128", name="ones128")
            nc.vector.memset(ones128[:], 1.0)
            # Warm the scalar engine's exp table while the PE runs
            # projections, so the first real exp doesn't pay the ~2.7us
            # ACT_TABLE_LOAD on the phase-2 critical path.
            warm = res.tile([P, 8], F32, tag="warm", name="warm")
            nc.vector.memset(warm[:], 0.0)
            nc.scalar.activation(warm[:], warm[:], AF.Exp, scale=1.0)

            def _rope(p0, p1, out0, out1, cs4, pool, tag):
                cos0, cos1, sin0, sin1 = cs4
                ta = pool.tile([P, 512], F32, tag="ropetmp", bufs=4,
                               name=f"ta{tag}")
                tb = pool.tile([P, 512], F32, tag="ropetmp", bufs=4,
                               name=f"tb{tag}")
                nc.vector.tensor_tensor(ta[:], p0[:], cos0, OP.mult)
                nc.vector.tensor_tensor(tb[:], p1[:], sin0, OP.mult)
                nc.vector.tensor_tensor(out0, ta[:], tb[:], OP.subtract)
                tc2 = pool.tile([P, 512], F32, tag="ropetmp", bufs=4,
                                name=f"tc{tag}")
                td = pool.tile([P, 512], F32, tag="ropetmp", bufs=4,
                               name=f"td{tag}")
                nc.vector.tensor_tensor(tc2[:], p1[:], cos1, OP.mult)
                nc.vector.tensor_tensor(td[:], p0[:], sin1, OP.mult)
                nc.vector.tensor_tensor(out1, tc2[:], td[:], OP.add)

            # ---------------- Phase 1: projections + RoPE ----------------
            with tc.tile_pool(name="proj", bufs=1) as proj, \
                 tc.tile_pool(name="proj_ps", space="PSUM", bufs=1) as pps:
                wkv_sb = proj.tile([P, 2 * EC * D], BF16, tag="wkv",
                                   name="wkv_sb")
                wqt_sb = proj.tile([P, EC * 512], BF16, tag="wqt",
                                   name="wqt_sb")
                xt_sb = proj.tile([P, 4 * EC * 512], BF16, tag="xt",
                                  name="xt_sb")
                cs_sb = proj.tile([P, 4 * 2048], BF16, tag="cs",
                                  name="cs_sb")

                def WK(e, i):
                    return wkv_sb[:, e * D + i * P:e * D + (i + 1) * P]

                def WV(e):
                    return wkv_sb[:, EC * D + e * D:EC * D + (e + 1) * D]

                def XS(lc, e):
                    base = lc * EC * 512 + e * 512
                    return xt_sb[:, base:base + 512]

                def WQ(e, j):
                    return wqt_sb[:, e * 512 + j * P:e * 512 + (j + 1) * P]

                def CS(lc, s, h):
                    base = lc * 2048 + s * 1024 + h * 512
                    return cs_sb[:, base:base + 512]

                # DMA issue order tracks PE consumption (Q first): wqt,
                # x^T lc0 in small chunks, cos/sin lc0, then wk/wv, then
                # the remaining lc blocks.
                for half in range(2):
                    hw = slice(half * EC * 256, (half + 1) * EC * 256)
                    nc.sync.dma_start(out=wqt_sb[:, hw], in_=wqt[:, hw])
                for ch in range(4):
                    csl = slice(ch * EC * 128, (ch + 1) * EC * 128)
                    nc.sync.dma_start(out=xt_sb[:, csl], in_=xt[:, csl])
                nc.sync.dma_start(out=cs_sb[:, 0:2048], in_=cs[:, 0:2048])
                nc.sync.dma_start(out=wkv_sb[:, 0:EC * D],
                                  in_=wkv[:, 0:EC * D])
                nc.sync.dma_start(out=wkv_sb[:, EC * D:2 * EC * D],
                                  in_=wkv[:, EC * D:2 * EC * D])
                for lc in range(1, 4):
                    lsl = slice(lc * EC * 512, (lc + 1) * EC * 512)
                    nc.sync.dma_start(out=xt_sb[:, lsl], in_=xt[:, lsl])
                    nc.sync.dma_start(
                        out=cs_sb[:, lc * 2048:(lc + 1) * 2048],
                        in_=cs[:, lc * 2048:(lc + 1) * 2048])

                # Per 512-column block: Q proj + RoPE, K proj + RoPE, V
                # proj.  Q-before-K keeps the final block's q-RoPE off the
                # phase-2 critical path: the last DVE work is the k-RoPE,
                # which overlaps the V projection.
                for lc in range(4):
                    sl = slice(lc * 512, (lc + 1) * 512)
                    cs4 = (CS(lc, 0, 0), CS(lc, 0, 1),
                           CS(lc, 1, 0), CS(lc, 1, 1))

                    pq = [pps.tile([P, 512], F32, tag="pq", bufs=4,
                                   name=f"pq{lc}_{j}") for j in range(4)]
                    for e in range(EC):
                        st, sp = (e == 0), (e == EC - 1)
                        xs = XS(lc, e)
                        for j in range(4):
                            nc.tensor.matmul(pq[j][:], WQ(e, j), xs,
                                             start=st, stop=sp)
                    _rope(pq[0], pq[1], qT[0][:, sl], qT[1][:, sl], cs4,
                          proj, f"q0{lc}")
                    _rope(pq[2], pq[3], qT[2][:, sl], qT[3][:, sl], cs4,
                          proj, f"q1{lc}")

                    pk0 = pps.tile([P, 512], F32, tag="pk", bufs=2,
                                   name=f"pk0_{lc}")
                    pk1 = pps.tile([P, 512], F32, tag="pk", bufs=2,
                                   name=f"pk1_{lc}")
                    for e in range(EC):
                        st, sp = (e == 0), (e == EC - 1)
                        xs = XS(lc, e)
                        nc.tensor.matmul(pk0[:], WK(e, 0), xs,
                                         start=st, stop=sp)
                        nc.tensor.matmul(pk1[:], WK(e, 1), xs,
                                         start=st, stop=sp)
                    _rope(pk0, pk1, kT[0][:, sl], kT[1][:, sl], cs4,
                          proj, f"k{lc}")

                    for lt in range(4 * lc, 4 * lc + 4):
                        pv = pps.tile([P, D], F32, tag="pv", bufs=2,
                                      name=f"pv{lt}")
                        for e in range(EC):
                            nc.tensor.matmul(
                                pv[:],
                                XS(lc, e)[:, (lt % 4) * P:(lt % 4 + 1) * P],
                                WV(e),
                                start=(e == 0), stop=(e == EC - 1))
                        nc.vector.tensor_copy(v_bf[lt][:], pv[:])

            # ------------- Phase 2: attention + o_proj -------------
            with tc.tile_pool(name="att", bufs=1) as att, \
                 tc.tile_pool(name="att_ps", space="PSUM", bufs=1) as aps:
                # G[hh][half]: gathered, normalized O^T.  G[c-row, col] with
                # c-row = d within half, column layout j*256 + r.
                G = [[att.tile([P, L], BF16, tag=f"G{hh}{dt}",
                               name=f"G{hh}{dt}") for dt in range(2)]
                     for hh in range(2)]
                wot_sb = att.tile([P, EC * E], BF16, tag="wot",
                                  name="wot_sb")
                for i in range(4):
                    qe = EC * E // 4
                    nc.sync.dma_start(out=wot_sb[:, i * qe:(i + 1) * qe],
                                      in_=wot[:, i * qe:(i + 1) * qe])

                def WOT(m, eg):
                    base = m * E + eg * 512
                    return wot_sb[:, base:base + 512]

                def o_proj(a_idx):
                    for rh in range(2):
                        rt = a_idx * 2 + rh
                        for eg in range(4):
                            py = aps.tile([P, 512], F32, tag="py", bufs=2,
                                          name=f"py{rt}_{eg}")
                            for m in range(EC):
                                lhsT = G[a_idx][m % 2][
                                    :, (m // 2) * 256 + rh * P:
                                       (m // 2) * 256 + rh * P + P]
                                nc.tensor.matmul(py[:], lhsT, WOT(m, eg),
                                                 start=(m == 0),
                                                 stop=(m == EC - 1))
                            ysb = att.tile([P, 512], F32, tag="ysb", bufs=3,
                                           name=f"ysb{rt}_{eg}")
                            nc.scalar.copy(ysb[:], py[:])
                            nc.sync.dma_start(
                                out=out[rt * P:(rt + 1) * P,
                                        eg * 512:(eg + 1) * 512],
                                in_=ysb[:])

                for hh in range(2):
                    qh0, qh1 = qT[2 * hh], qT[2 * hh + 1]
                    for lqc in range(4):
                        qsl = slice(lqc * 512, (lqc + 1) * 512)
                        pt = [att.tile([P, 512], BF16, tag=f"pt{i}", bufs=2,
                                       name=f"pt{hh}_{lqc}_{i}")
                              for i in range(LT)]
                        for lk in range(LT):
                            ps = aps.tile([P, 512], F32, tag="ps", bufs=3,
                                          name=f"ps{hh}_{lqc}_{lk}")
                            nc.tensor.matmul(ps[:],
                                             kT[0][:, lk * P:(lk + 1) * P],
                                             qh0[:, qsl],
                                             start=True, stop=False)
                            nc.tensor.matmul(ps[:],
                                             kT[1][:, lk * P:(lk + 1) * P],
                                             qh1[:, qsl],
                                             start=False, stop=True)
                            nc.scalar.activation(pt[lk][:], ps[:], AF.Exp,
                                                 scale=float(SCALING))
                        # Pair-reduce the 16 P^T tiles twice on DVE (bf16),
                        # so the cross-partition row-sum matmul streams 4
                        # tiles instead of 16.
                        s8 = [att.tile([P, 512], BF16, tag=f"s8_{i}",
                                       bufs=2, name=f"s8{hh}_{lqc}_{i}")
                              for i in range(8)]
                        for i in range(8):
                            nc.vector.tensor_tensor(s8[i][:], pt[2 * i][:],
                                                    pt[2 * i + 1][:], OP.add)
                        s4 = [att.tile([P, 512], BF16, tag=f"s4_{i}",
                                       bufs=2, name=f"s4{hh}_{lqc}_{i}")
                              for i in range(4)]
                        for i in range(4):
                            nc.vector.tensor_tensor(s4[i][:], s8[2 * i][:],
                                                    s8[2 * i + 1][:], OP.add)
                        # attn @ v first (keeps PE busy while DVE finishes
                        # the pair-adds), then the row-sum matmul.
                        po = [None, None]
                        for dt in range(2):
                            po[dt] = aps.tile([P, 512], F32, tag="po",
                                              bufs=2,
                                              name=f"po{hh}_{lqc}_{dt}")
                            for lk in range(LT):
                                nc.tensor.matmul(
                                    po[dt][:],
                                    v_bf[lk][:, dt * P:(dt + 1) * P],
                                    pt[lk][:],
                                    start=(lk == 0), stop=(lk == LT - 1))
                        # Row sums, pre-broadcast over all 128 partitions
                        # by the all-ones stationary operand.
                        prb = aps.tile([P, 512], F32, tag="prb", bufs=1,
                                       name=f"prb{hh}_{lqc}")
                        for i in range(4):
                            nc.tensor.matmul(prb[:], ones128[:], s4[i][:],
                                             start=(i == 0), stop=(i == 3))
                        rb = att.tile([P, 512], F32, tag="rb", bufs=2,
                                      name=f"rb{hh}_{lqc}")
                        nc.vector.reciprocal_approx_fast(rb[:], prb[:])
                        rb_wu = rb.rearrange("p (u w) -> p w u", w=8)
                        for dt in range(2):
                            # normalize + ColPali gather in one op:
                            # G[:, j*256 + 64*lqc + u] = po[:, 8u+j]*rb[:, 8u+j]
                            g_dst = G[hh][dt].rearrange(
                                "p (w r) -> p w r",
                                w=8)[:, :, 64 * lqc:64 * lqc + 64]
                            nc.vector.tensor_tensor(
                                g_dst,
                                po[dt].rearrange("p (u w) -> p w u", w=8),
                                rb_wu, OP.mult)
                    o_proj(hh)

    nc.compile()
    return nc


_NC = None


def _get_nc():
    global _NC
    if _NC is None:
        _NC = build_program()
    return _NC


def _pack(a):
    """[nb*128, C] -> [128, nb*C], block-major: out[p, i*C+j] = a[i*128+p, j]."""
    R, C = a.shape
    nb = R // P
    return np.ascontiguousarray(
        a.reshape(nb, P, C).transpose(1, 0, 2).reshape(P, nb * C))


def make_in_maps(hidden_states, cos, sin, Wq, Wk, Wv, Wo):
    bf = ml_dtypes.bfloat16
    hs = np.asarray(hidden_states, np.float32)
    cosT = np.asarray(cos, np.float32).T.astype(bf)  # [D, L]
    sinT = np.asarray(sin, np.float32).T.astype(bf)
    xtb = [hs[b].T.astype(bf) for b in range(B)]     # [E, L]
    # xt packed: [128, lc*8192 + e*512 + j]
    xt_p = [np.concatenate(
        [_pack(x[:, lc * 512:(lc + 1) * 512]) for lc in range(4)], axis=1)
        for x in xtb]
    # cs packed: [128, lc*2048 + s*1024 + h*512 + j]
    cs_p = np.concatenate(
        [_pack(src[:, lc * 512:(lc + 1) * 512])
         for lc in range(4) for src in (cosT, sinT)], axis=1)
    wkv_p = np.concatenate(
        [_pack(np.asarray(Wk, np.float32).T.astype(bf)),
         _pack(np.asarray(Wv, np.float32).T.astype(bf))], axis=1)
    wqT = np.asarray(Wq, np.float32).T.astype(bf)    # [E, HD]
    wot_p = _pack(np.asarray(Wo, np.float32).T.astype(bf))
    in_maps = []
    for c in range(N_CORES):
        b, ql = c // 4, c % 4
        sl = slice(ql * 512, (ql + 1) * 512)
        in_maps.append({
            "xt": xt_p[b],
            "cs": cs_p,
            "wkv": wkv_p,
            "wqt": _pack(np.ascontiguousarray(wqT[:, sl])),
            "wot": wot_p,
        })
    return in_maps


def assemble(results):
    y = np.empty((B, L, E), np.float32)
    for c in range(N_CORES):
        b, ql = c // 4, c % 4
        y[b, ql * 512:(ql + 1) * 512, :] = results[c]["out"]
    return y


def kernel(hidden_states, attention_mask, cos, sin, Wq, Wk, Wv, Wo):
    # attention_mask is additive and all-zero per the problem spec; it is
    # accepted for signature compatibility but not shipped to the device.
    nc = _get_nc()
    in_maps = make_in_maps(hidden_states, cos, sin, Wq, Wk, Wv, Wo)
    res = run_bass_kernel_spmd(nc, in_maps, core_ids=list(range(N_CORES)))
    return assemble(res.results)


# revision 22
# speedup vs baseline: 1.2198x; 1.0304x over previous
"""ColPali MQA attention block on 8 Trainium2 NeuronCores.

The reference contains the ColPali reshape quirk: the attention output
[B, H, L, 1, D] is reshaped row-major straight to [B, L, H*D], which mixes
heads and positions.  Output row l' therefore depends ONLY on head
h = l'//256, gathering positions (l'%256)*8 + j for j in 0..7:

    Y[b, l', e] = sum_{j,d} O[b, l'//256, (l'%256)*8+j, d] * Wo[e, j*256+d]

Sharding: core c -> batch b=c//4 and heads {h0, h0+1} with h0=2*(c%4).
Each core computes K/V projection for its batch (replicated inside the
4-core batch group), Q projection + attention for its 2 heads over the full
sequence, and o_proj for output rows [512*(c%4), +512).  Per-core outputs
are disjoint [512, 2048] slices of the [2, 2048, 2048] output -> no
cross-core communication.  (A K/V AllGather variant was measured: it saves
~50us of PE work but its SDMA burst trips the chip's power throttler
(K=13/16 for the next ~200us firmware period), costing more than it saves.)

All inputs are pre-cast to bf16 AND pre-packed tile-major on the host, so
every device DMA is one fat contiguous transfer (2-16KB per partition
line) straight into its SBUF-resident layout -- no staging, no on-device
conversion, no small-line DMA inefficiency.

Layouts (contraction dim always on SBUF partitions; zero on-device
transposes):
  - q, k produced transposed ([D, L]) by making W the stationary operand.
  - v produced natural ([L, D]) by making X the stationary operand.
  - K/V/Q projections interleaved per 512-column block of x^T so the PE
    ramps as blocks arrive and psum WAR hazards never stall.
  - scores computed transposed: S^T[lk, lq] = k @ q_h^T, so the exp output
    P^T[lk, lq] directly feeds O^T[d, lq] = v^T @ P^T as moving operand.
  - softmax row sums over lk (= partitions): the 16 P^T tiles are first
    pair-reduced twice on the vector engine (bf16), then a single all-ones
    [128,128] stationary matmul pass over the 4 partial tiles lands the
    sums pre-broadcast across all 128 partitions; reciprocal_approx_fast
    gives the scale tile.  The normalize multiply writes through a
    (u w)->(w u) access pattern that performs the ColPali gather for free,
    producing G[c, r] = O^T[d, r*8+j] (c = j*256+d) which is directly the
    stationary operand of o_proj.
"""

import numpy as np
import ml_dtypes

import concourse.mybir as mybir
import concourse.tile as tile
from concourse import bacc
from concourse.bass_utils import run_bass_kernel_spmd

F32 = mybir.dt.float32
BF16 = mybir.dt.bfloat16
AF = mybir.ActivationFunctionType
OP = mybir.AluOpType

B, L, H, D, E = 2, 2048, 8, 256, 2048
HD = H * D  # 2048
P = 128
EC = E // P  # 16 e-chunks
LT = L // P  # 16 l-tiles
SCALING = D ** -0.5  # 1/16
N_CORES = 8


def build_program():
    nc = bacc.Bacc("TRN2", target_bir_lowering=False, debug=False,
                   num_devices=N_CORES)

    # Tile-major packed inputs (see make_in_maps for the host-side layout).
    xt = nc.dram_tensor("xt", [P, 4 * EC * 512], BF16,
                        kind="ExternalInput").ap()
    cs = nc.dram_tensor("cs", [P, 4 * 2048], BF16, kind="ExternalInput").ap()
    wkv = nc.dram_tensor("wkv", [P, 2 * EC * D], BF16,
                         kind="ExternalInput").ap()
    wqt = nc.dram_tensor("wqt", [P, EC * 512], BF16,
                         kind="ExternalInput").ap()
    wot = nc.dram_tensor("wot", [P, EC * E], BF16, kind="ExternalInput").ap()
    out = nc.dram_tensor("out", [4 * P, E], F32, kind="ExternalOutput").ap()

    with tile.TileContext(nc) as tc:
        with tc.tile_pool(name="res", bufs=1) as res:
            kT = [res.tile([P, L], BF16, tag=f"kT{i}", name=f"kT{i}")
                  for i in range(2)]
            v_bf = [res.tile([P, D], BF16, tag=f"v{i}", name=f"v{i}")
                    for i in range(LT)]
            # q^T for the core's two heads: 4 dq-tiles x [128, L]
            qT = [res.tile([P, L], BF16, tag=f"qT{i}", name=f"qT{i}")
                  for i in range(4)]
            ones128 = res.tile([P, P], BF16, tag="ones128", name="ones128")
            nc.vector.memset(ones128[:], 1.0)
            # Warm the scalar engine's exp table while the PE runs
            # projections, so the first real exp doesn't pay the ~2.7us
            # ACT_TABLE_LOAD on the phase-2 critical path.
            warm = res.tile([P, 8], F32, tag="warm", name="warm")
            nc.vector.memset(warm[:], 0.0)
            nc.scalar.activation(warm[:], warm[:], AF.Exp, scale=1.0)

            def _rope(p0, p1, out0, out1, cs4, pool, tag):
                cos0, cos1, sin0, sin1 = cs4
                ta = pool.tile([P, 512], F32, tag="ropetmp", bufs=4,
                               name=f"ta{tag}")
                tb = pool.tile([P, 512], F32, tag="ropetmp", bufs=4,
                               name=f"tb{tag}")
                nc.vector.tensor_tensor(ta[:], p0[:], cos0, OP.mult)
                nc.vector.tensor_tensor(tb[:], p1[:], sin0, OP.mult)
                nc.vector.tensor_tensor(out0, ta[:], tb[:], OP.subtract)
                tc2 = pool.tile([P, 512], F32, tag="ropetmp", bufs=4,
                                name=f"tc{tag}")
                td = pool.tile([P, 512], F32, tag="ropetmp", bufs=4,
                               name=f"td{tag}")
                nc.vector.tensor_tensor(tc2[:], p1[:], cos1, OP.mult)
                nc.vector.tensor_tensor(td[:], p0[:], sin1, OP.mult)
                nc.vector.tensor_tensor(out1, tc2[:], td[:], OP.add)

            # ---------------- Phase 1: projections + RoPE ----------------
            with tc.tile_pool(name="proj", bufs=1) as proj, \
                 tc.tile_pool(name="proj_ps", space="PSUM", bufs=1) as pps:
                wkv_sb = proj.tile([P, 2 * EC * D], BF16, tag="wkv",
                                   name="wkv_sb")
                wqt_sb = proj.tile([P, EC * 512], BF16, tag="wqt",
                                   name="wqt_sb")
                xt_sb = proj.tile([P, 4 * EC * 512], BF16, tag="xt",
                                  name="xt_sb")
                cs_sb = proj.tile([P, 4 * 2048], BF16, tag="cs",
                                  name="cs_sb")

                def WK(e, i):
                    return wkv_sb[:, e * D + i * P:e * D + (i + 1) * P]

                def WV(e):
                    return wkv_sb[:, EC * D + e * D:EC * D + (e + 1) * D]

                def XS(lc, e):
                    base = lc * EC * 512 + e * 512
                    return xt_sb[:, base:base + 512]

                def WQ(e, j):
                    return wqt_sb[:, e * 512 + j * P:e * 512 + (j + 1) * P]

                def CS(lc, s, h):
                    base = lc * 2048 + s * 1024 + h * 512
                    return cs_sb[:, base:base + 512]

                # DMA issue order tracks PE consumption (Q first): wqt,
                # x^T lc0 in small chunks, cos/sin lc0, then wk/wv, then
                # the remaining lc blocks.
                for ch in range(4):
                    qw = slice(ch * EC * 128, (ch + 1) * EC * 128)
                    nc.sync.dma_start(out=wqt_sb[:, qw], in_=wqt[:, qw])
                    nc.sync.dma_start(out=xt_sb[:, qw], in_=xt[:, qw])
                nc.sync.dma_start(out=cs_sb[:, 0:2048], in_=cs[:, 0:2048])
                nc.sync.dma_start(out=wkv_sb[:, 0:EC * D],
                                  in_=wkv[:, 0:EC * D])
                nc.sync.dma_start(out=wkv_sb[:, EC * D:2 * EC * D],
                                  in_=wkv[:, EC * D:2 * EC * D])
                for lc in range(1, 4):
                    lsl = slice(lc * EC * 512, (lc + 1) * EC * 512)
                    nc.sync.dma_start(out=xt_sb[:, lsl], in_=xt[:, lsl])
                    nc.sync.dma_start(
                        out=cs_sb[:, lc * 2048:(lc + 1) * 2048],
                        in_=cs[:, lc * 2048:(lc + 1) * 2048])

                # Per 512-column block: Q proj + RoPE, K proj + RoPE, V
                # proj.  Q-before-K keeps the final block's q-RoPE off the
                # phase-2 critical path: the last DVE work is the k-RoPE,
                # which overlaps the V projection.
                for lc in range(4):
                    sl = slice(lc * 512, (lc + 1) * 512)
                    cs4 = (CS(lc, 0, 0), CS(lc, 0, 1),
                           CS(lc, 1, 0), CS(lc, 1, 1))

                    pq = [pps.tile([P, 512], F32, tag="pq", bufs=4,
                                   name=f"pq{lc}_{j}") for j in range(4)]
                    for e in range(EC):
                        st, sp = (e == 0), (e == EC - 1)
                        xs = XS(lc, e)
                        for j in range(4):
                            nc.tensor.matmul(pq[j][:], WQ(e, j), xs,
                                             start=st, stop=sp)
                    _rope(pq[0], pq[1], qT[0][:, sl], qT[1][:, sl], cs4,
                          proj, f"q0{lc}")
                    _rope(pq[2], pq[3], qT[2][:, sl], qT[3][:, sl], cs4,
                          proj, f"q1{lc}")

                    pk0 = pps.tile([P, 512], F32, tag="pk", bufs=2,
                                   name=f"pk0_{lc}")
                    pk1 = pps.tile([P, 512], F32, tag="pk", bufs=2,
                                   name=f"pk1_{lc}")
                    for e in range(EC):
                        st, sp = (e == 0), (e == EC - 1)
                        xs = XS(lc, e)
                        nc.tensor.matmul(pk0[:], WK(e, 0), xs,
                                         start=st, stop=sp)
                        nc.tensor.matmul(pk1[:], WK(e, 1), xs,
                                         start=st, stop=sp)
                    _rope(pk0, pk1, kT[0][:, sl], kT[1][:, sl], cs4,
                          proj, f"k{lc}")

                    for lt in range(4 * lc, 4 * lc + 4):
                        pv = pps.tile([P, D], F32, tag="pv", bufs=2,
                                      name=f"pv{lt}")
                        for e in range(EC):
                            nc.tensor.matmul(
                                pv[:],
                                XS(lc, e)[:, (lt % 4) * P:(lt % 4 + 1) * P],
                                WV(e),
                                start=(e == 0), stop=(e == EC - 1))
                        nc.vector.tensor_copy(v_bf[lt][:], pv[:])

            # ------------- Phase 2: attention + o_proj -------------
            with tc.tile_pool(name="att", bufs=1) as att, \
                 tc.tile_pool(name="att_ps", space="PSUM", bufs=1) as aps:
                # G[hh][half]: gathered, normalized O^T.  G[c-row, col] with
                # c-row = d within half, column layout j*256 + r.
                G = [[att.tile([P, L], BF16, tag=f"G{hh}{dt}",
                               name=f"G{hh}{dt}") for dt in range(2)]
                     for hh in range(2)]
                wot_sb = att.tile([P, EC * E], BF16, tag="wot",
                                  name="wot_sb")
                for i in range(4):
                    qe = EC * E // 4
                    nc.sync.dma_start(out=wot_sb[:, i * qe:(i + 1) * qe],
                                      in_=wot[:, i * qe:(i + 1) * qe])

                def WOT(m, eg):
                    base = m * E + eg * 512
                    return wot_sb[:, base:base + 512]

                def o_proj(a_idx):
                    for rh in range(2):
                        rt = a_idx * 2 + rh
                        for eg in range(4):
                            py = aps.tile([P, 512], F32, tag="py", bufs=2,
                                          name=f"py{rt}_{eg}")
                            for m in range(EC):
                                lhsT = G[a_idx][m % 2][
                                    :, (m // 2) * 256 + rh * P:
                                       (m // 2) * 256 + rh * P + P]
                                nc.tensor.matmul(py[:], lhsT, WOT(m, eg),
                                                 start=(m == 0),
                                                 stop=(m == EC - 1))
                            ysb = att.tile([P, 512], F32, tag="ysb", bufs=3,
                                           name=f"ysb{rt}_{eg}")
                            nc.scalar.copy(ysb[:], py[:])
                            nc.sync.dma_start(
                                out=out[rt * P:(rt + 1) * P,
                                        eg * 512:(eg + 1) * 512],
                                in_=ysb[:])

                for hh in range(2):
                    qh0, qh1 = qT[2 * hh], qT[2 * hh + 1]
                    for lqc in range(4):
                        qsl = slice(lqc * 512, (lqc + 1) * 512)
                        pt = [att.tile([P, 512], BF16, tag=f"pt{i}", bufs=2,
                                       name=f"pt{hh}_{lqc}_{i}")
                              for i in range(LT)]
                        for lk in range(LT):
                            ps = aps.tile([P, 512], F32, tag="ps", bufs=3,
                                          name=f"ps{hh}_{lqc}_{lk}")
                            nc.tensor.matmul(ps[:],
                                             kT[0][:, lk * P:(lk + 1) * P],
                                             qh0[:, qsl],
                                             start=True, stop=False)
                            nc.tensor.matmul(ps[:],
                                             kT[1][:, lk * P:(lk + 1) * P],
                                             qh1[:, qsl],
                                             start=False, stop=True)
                            nc.scalar.activation(pt[lk][:], ps[:], AF.Exp,
                                                 scale=float(SCALING))
                        # Pair-reduce the 16 P^T tiles twice on DVE (bf16),
                        # so the cross-partition row-sum matmul streams 4
                        # tiles instead of 16.
                        s8 = [att.tile([P, 512], BF16, tag=f"s8_{i}",
                                       bufs=2, name=f"s8{hh}_{lqc}_{i}")
                              for i in range(8)]
                        for i in range(8):
                            nc.vector.tensor_tensor(s8[i][:], pt[2 * i][:],
                                                    pt[2 * i + 1][:], OP.add)
                        s4 = [att.tile([P, 512], BF16, tag=f"s4_{i}",
                                       bufs=2, name=f"s4{hh}_{lqc}_{i}")
                              for i in range(4)]
                        for i in range(4):
                            nc.vector.tensor_tensor(s4[i][:], s8[2 * i][:],
                                                    s8[2 * i + 1][:], OP.add)
                        s2 = [att.tile([P, 512], BF16, tag=f"s2_{i}",
                                       bufs=2, name=f"s2{hh}_{lqc}_{i}")
                              for i in range(2)]
                        for i in range(2):
                            nc.vector.tensor_tensor(s2[i][:], s4[2 * i][:],
                                                    s4[2 * i + 1][:], OP.add)
                        s1 = att.tile([P, 512], BF16, tag="s1", bufs=2,
                                      name=f"s1{hh}_{lqc}")
                        nc.vector.tensor_tensor(s1[:], s2[0][:], s2[1][:],
                                                OP.add)
                        # attn @ v first (keeps PE busy while DVE finishes
                        # the pair-adds), then the row-sum matmul.
                        po = [None, None]
                        for dt in range(2):
                            po[dt] = aps.tile([P, 512], F32, tag="po",
                                              bufs=2,
                                              name=f"po{hh}_{lqc}_{dt}")
                            for lk in range(LT):
                                nc.tensor.matmul(
                                    po[dt][:],
                                    v_bf[lk][:, dt * P:(dt + 1) * P],
                                    pt[lk][:],
                                    start=(lk == 0), stop=(lk == LT - 1))
                        # Row sums, pre-broadcast over all 128 partitions
                        # by the all-ones stationary operand.
                        prb = aps.tile([P, 512], F32, tag="prb", bufs=1,
                                       name=f"prb{hh}_{lqc}")
                        nc.tensor.matmul(prb[:], ones128[:], s1[:],
                                         start=True, stop=True)
                        rb = att.tile([P, 512], F32, tag="rb", bufs=2,
                                      name=f"rb{hh}_{lqc}")
                        nc.vector.reciprocal_approx_fast(rb[:], prb[:])
                        rb_wu = rb.rearrange("p (u w) -> p w u", w=8)
                        for dt in range(2):
                            # normalize + ColPali gather in one op:
                            # G[:, j*256 + 64*lqc + u] = po[:, 8u+j]*rb[:, 8u+j]
                            g_dst = G[hh][dt].rearrange(
                                "p (w r) -> p w r",
                                w=8)[:, :, 64 * lqc:64 * lqc + 64]
                            nc.vector.tensor_tensor(
                                g_dst,
                                po[dt].rearrange("p (u w) -> p w u", w=8),
                                rb_wu, OP.mult)
                    o_proj(hh)

    nc.compile()
    return nc


_NC = None


def _get_nc():
    global _NC
    if _NC is None:
        _NC = build_program()
    return _NC


def _pack(a):
    """[nb*128, C] -> [128, nb*C], block-major: out[p, i*C+j] = a[i*128+p, j]."""
    R, C = a.shape
    nb = R // P
    return np.ascontiguousarray(
        a.reshape(nb, P, C).transpose(1, 0, 2).reshape(P, nb * C))


def make_in_maps(hidden_states, cos, sin, Wq, Wk, Wv, Wo):
    bf = ml_dtypes.bfloat16
    hs = np.asarray(hidden_states, np.float32)
    cosT = np.asarray(cos, np.float32).T.astype(bf)  # [D, L]
    sinT = np.asarray(sin, np.float32).T.astype(bf)
    xtb = [hs[b].T.astype(bf) for b in range(B)]     # [E, L]
    # xt packed: [128, lc*8192 + e*512 + j]
    xt_p = [np.concatenate(
        [_pack(x[:, lc * 512:(lc + 1) * 512]) for lc in range(4)], axis=1)
        for x in xtb]
    # cs packed: [128, lc*2048 + s*1024 + h*512 + j]
    cs_p = np.concatenate(
        [_pack(src[:, lc * 512:(lc + 1) * 512])
         for lc in range(4) for src in (cosT, sinT)], axis=1)
    wkv_p = np.concatenate(
        [_pack(np.asarray(Wk, np.float32).T.astype(bf)),
         _pack(np.asarray(Wv, np.float32).T.astype(bf))], axis=1)
    wqT = np.asarray(Wq, np.float32).T.astype(bf)    # [E, HD]
    wot_p = _pack(np.asarray(Wo, np.float32).T.astype(bf))
    in_maps = []
    for c in range(N_CORES):
        b, ql = c // 4, c % 4
        sl = slice(ql * 512, (ql + 1) * 512)
        in_maps.append({
            "xt": xt_p[b],
            "cs": cs_p,
            "wkv": wkv_p,
            "wqt": _pack(np.ascontiguousarray(wqT[:, sl])),
            "wot": wot_p,
        })
    return in_maps


def assemble(results):
    y = np.empty((B, L, E), np.float32)
    for c in range(N_CORES):
        b, ql = c // 4, c % 4
        y[b, ql * 512:(ql + 1) * 512, :] = results[c]["out"]
    return y


def kernel(hidden_states, attention_mask, cos, sin, Wq, Wk, Wv, Wo):
    # attention_mask is additive and all-zero per the problem spec; it is
    # accepted for signature compatibility but not shipped to the device.
    nc = _get_nc()
    in_maps = make_in_maps(hidden_states, cos, sin, Wq, Wk, Wv, Wo)
    res = run_bass_kernel_spmd(nc, in_maps, core_ids=list(range(N_CORES)))
    return assemble(res.results)
